# revision 75
# baseline (speedup 1.0000x reference)
"""Trainium2 Bass kernel for 8-head MultiHeadAttention (B=2, S=4096, E=512).

Sharding: 8 cores = 2 batches x 4 query-row chunks of 1024; each core runs
all 8 heads for its (batch, q-range) as 4 head-pairs x 2 query-windows, with
the k-dimension processed in 32 chunks of 128.

v2 design (vs v0):
- K-projection is folded into Q on the host: G = Wq^T @ Wk, so
  scores = (x_q G) . x_k^T and the raw (f16) K tiles are the score matmul's
  stationary operand directly. Only a tiny Q-side projection remains
  (1 blockdiag matmul per 512-col chunk). Wv stays folded into Wo.
- Whole 16-bit pipeline is f16 (x, qp, pt, V, ctx, Wo) - f16's 10-bit
  mantissa keeps the base quantization error ~6e-4, leaving the error
  budget to the Schraudolph trick.
- exp via f16-bits Schraudolph on DVE *and* Pool (both support
  scalar_tensor_tensor): ONE op computes i16 = round((s + 82.93) *
  (184.5 * mask)) whose bits ARE the f16 weights; mask folded in as the
  multiplicand. 22 of 32 k-chunks go this way; 10 use the ACT table exp
  (f16 out) with a post-exp f16 mask multiply on Pool/DVE.
- PE only does matmuls: scores (f16, 512 rows), attention-value flip
  (pt stationary, V+ones moving, 65 rows), q-proj, transposes, out-proj.
  No mask work on PE.
- cx accumulators bank-aligned in PSUM; AV runs 8 half-units behind
  scores; transposes/out-proj trickle into later attention blocks.
"""
import sys
for _p in ('/root/.axon_site/_ro/trn_rl_repo', '/opt/trn_rl_repo'):
    if _p not in sys.path:
        sys.path.append(_p)

import numpy as np
import ml_dtypes

import concourse.bass as bass
import concourse.tile as tile
from concourse import bacc, mybir
from concourse import bass_utils

F32 = mybir.dt.float32
F16 = mybir.dt.float16
BF16 = mybir.dt.bfloat16
FP8 = mybir.dt.float8e4
I16 = mybir.dt.int16
AF = mybir.ActivationFunctionType
ALU = mybir.AluOpType

N_CORES = 8
B, S, E, H, DH = 2, 4096, 512, 8, 64
QLEN = S // 4           # 1024 q rows per core
KC = S // 128           # 32 k chunks

# f16-bits Schraudolph: i16 = round((s + BB) * (A16 * mask)); the i16 bit
# pattern read as f16 is ~exp(s/8). A16 = 1024*log2(e)/8 rounded to an
# f16-exact value; BB calibrated so the mean weight ratio vs exact exp is 1.
# The saw-chunk mask is stored fp8 as m*176 (e4m3-exact); the 184.5/176
# ratio is folded into G on the host (scores come out scaled by 184.5/176),
# so bits = (s' + BB8) * (176 * m) == (s + BB) * (184.5 * m) exactly.
# bf16-bits variant: the dataset's extreme scores (|s| up to ~92, heavy
# product-normal tails) give weights up to e^11.4 = 9e4 > f16 max, so the
# weights (pt) live in bf16. bits = (QSCALE*s + BB8) * (24 * m): 24 is
# e4m3-exact, QSCALE = (128*log2(e)/8)/24 is folded into G on the host,
# BB8 calibrated for mean weight ratio 1 (rms sawtooth ~1.78%).
ABITS = 23.083120654223414      # 128*log2(e)/8
V8 = 24.0                       # fp8 mask multiplier
QSCALE = ABITS / V8             # host folds this into G
BB8 = 677.026428
EXPSCALE = 0.125 / QSCALE       # ACT exp scale on the rescaled scores

# per-kc class: 'saw' = Schraudolph stt (mask folded, on DVE or Pool);
# 'act' = ACT table exp (f16) + post-exp mask multiply on DVE.
# per-kc classes (GPSIMD cannot touch PSUM, so exp lives on ACT+DVE only):
#  'saw': DVE Schraudolph stt, mask folded (fp8 m*176 operand)
#  'act': ACT table exp + post-exp f16 mask multiply (DVE or Pool, SBUF)
#  'pe':  ACT table exp, mask pre-added into the score PSUM by an fp8
#         DoubleRow identity matmul (i2=eye*1.5 against the m*176 fp8 mask
#         adds 264*m; exp bias -264*EXPSCALE turns masked scores into ~e-31)
# period-8 layout keeps runs short for engine smoothness and mask DMAs
# groupable by dtype run (mb8 for saw+pe, f16 for act).
# act = all odd kcs (ACT-exp runs stay <= 3 incl. the pe islands); pe on 4
# spread evens; saw on the other 12 evens. Odd/even interleave keeps every
# engine's per-unit load smooth AND leaves each dtype's chunks on an
# affine stride-2 grid so mask DMAs still group (4 chunks per transfer).
ACT_KCS = set(range(1, KC, 2))
PE_KCS = {4, 12, 20, 28}
CLS = {kc: ('act' if kc % 2 else ('pe' if kc in PE_KCS else 'saw'))
       for kc in range(KC)}
# (c0, n_chunks) with chunk stride 2; dtype = f16 for odd c0, fp8 for even
MASK_GROUPS = [(0, 4), (1, 4), (8, 4), (9, 4), (16, 4), (17, 4),
               (24, 4), (25, 4)]
VALP_RUNS = [(0, 1), (1, 1), (2, 2), (4, 2), (6, 2), (8, 4), (12, 4),
             (16, 4), (20, 4), (24, 4), (28, 4)]

# act-chunk mask multiplies alternate DVE / Pool (both SBUF-legal; Pool
# runs TT at 0.42x roofline so it only takes ~60%)
MSK_CYCLE = ('d', 'd', 'p', 'd', 'd', 'p', 'd', 'p')

_CACHE = {}


def _build_module():
    nc = bacc.Bacc("TRN2", target_bir_lowering=False, debug=False,
                   enable_asserts=True, num_devices=N_CORES)

    xkT = nc.dram_tensor("xkT", [E, S], F16, kind="ExternalInput").ap()
    xqT = nc.dram_tensor("xqT", [E, QLEN], F16, kind="ExternalInput").ap()
    valp = nc.dram_tensor("valp", [S, H * 65], BF16, kind="ExternalInput").ap()
    mbx = nc.dram_tensor("mbx", [S, QLEN], BF16, kind="ExternalInput").ap()
    mb8 = nc.dram_tensor("mb8", [S, QLEN], FP8, kind="ExternalInput").ap()
    g2 = nc.dram_tensor("g2", [128, 128], F16, kind="ExternalInput").ap()
    woe = nc.dram_tensor("woe", [E, E], F16, kind="ExternalInput").ap()
    bo_b = nc.dram_tensor("bo_b", [128, E], F32, kind="ExternalInput").ap()
    eye_d = nc.dram_tensor("eye", [128, 128], F16, kind="ExternalInput").ap()
    i2_d = nc.dram_tensor("i2", [128, 256], FP8, kind="ExternalInput").ap()
    out = nc.dram_tensor("out", [QLEN, E], F32, kind="ExternalOutput").ap()

    with tile.TileContext(nc) as tc:
        _emit(tc, nc, xkT, xqT, valp, mbx, mb8, g2, woe, bo_b, eye_d, i2_d,
              out)

    nc.compile()
    return nc


def _emit(tc, nc, xkT, xqT, valp, mbx, mb8, g2, woe, bo_b, eye_d, i2_d, out):
    from contextlib import ExitStack
    ctx = ExitStack()
    const = ctx.enter_context(tc.tile_pool(name="const", bufs=1))
    kqp = ctx.enter_context(tc.tile_pool(name="kqp", bufs=1))
    xst = ctx.enter_context(tc.tile_pool(name="xst", bufs=2))
    ptp = ctx.enter_context(tc.tile_pool(name="pt", bufs=8))
    ctn_p = ctx.enter_context(tc.tile_pool(name="ctn", bufs=5))
    osb_p = ctx.enter_context(tc.tile_pool(name="osb", bufs=4))
    psb = ctx.enter_context(tc.tile_pool(name="psb", bufs=3, space="PSUM"))
    ctxp = ctx.enter_context(tc.tile_pool(name="ctxp", bufs=2, space="PSUM"))

    # ---------------- constants ----------------
    g2_sb = const.tile([128, 128], F16, tag="g2")
    nc.sync.dma_start(g2_sb, g2)
    eye = const.tile([128, 128], F16, tag="eye")
    nc.sync.dma_start(eye, eye_d)
    i2 = const.tile([128, 256], FP8, tag="i2")
    nc.sync.dma_start(i2, i2_d)
    woe_sb = [const.tile([128, E], F16, tag=f"woe{pc}", name=f"woe{pc}")
              for pc in range(4)]
    bo_sb = const.tile([128, E], F32, tag="bo")
    biasB = const.tile([128, 1], F32, tag="biasB")
    nc.vector.memset(biasB, 0.0)
    biasC = const.tile([128, 1], F32, tag="biasC")
    nc.vector.memset(biasC, -240.0 * EXPSCALE)

    def load_late_consts():
        for pc in range(4):
            nc.sync.dma_start(woe_sb[pc], woe[pc * 128:(pc + 1) * 128, :])
        nc.sync.dma_start(bo_sb, bo_b)

    # resident masks + V: one run-tile per contiguous same-class chunk run,
    # loaded by ONE grouped DMA each (HWDGE desc-gen would otherwise
    # serialize 64 transfers at ~600ns apiece). saw-chunk masks are fp8
    # (m*176); act-chunk masks f16 (keeps the DVE TT in 2x mode).
    mbx_res = {}
    mask_run_t = {}
    for (c0, k) in MASK_GROUPS:
        dt = FP8 if CLS[c0] != 'act' else BF16
        rt = const.tile([128, k * QLEN], dt, tag=f"mr{c0}", name=f"mr{c0}")
        mask_run_t[(c0, k)] = rt
        for j in range(k):
            mbx_res[c0 + 2 * j] = rt[:, j * QLEN:(j + 1) * QLEN]
    valp_run_t = {}
    valp_t = [None] * KC
    for (c0, k) in VALP_RUNS:
        rt = const.tile([128, k * H * 65], BF16, tag=f"vr{c0}", name=f"vr{c0}")
        valp_run_t[(c0, k)] = rt
        for j in range(k):
            valp_t[c0 + j] = rt[:, j * H * 65:(j + 1) * H * 65]

    def _grouped_dma(dst, src_ap, c0, k, row_elems):
        src = bass.AP(tensor=src_ap.tensor,
                      offset=src_ap.offset + c0 * 128 * row_elems,
                      ap=[[row_elems, 128], [128 * row_elems, k],
                          [1, row_elems]])
        nc.sync.dma_start(dst, src)

    def _mask_half_dma(c0, k, half):
        # load q-columns [half*512, half*512+512) of k stride-2 chunks in
        # one DMA: block 0 only touches the qw=0 halves, so splitting keeps
        # the critical first-block stream under its consumption rate.
        rt = mask_run_t[(c0, k)]
        src_ap = mb8 if CLS[c0] != 'act' else mbx
        dst = bass.AP(tensor=rt.tensor, offset=rt.offset + half * 512,
                      ap=[rt.ap[0], [QLEN, k], [1, 512]])
        src = bass.AP(tensor=src_ap.tensor,
                      offset=src_ap.offset + c0 * 128 * QLEN + half * 512,
                      ap=[[QLEN, 128], [2 * 128 * QLEN, k], [1, 512]])
        nc.sync.dma_start(dst, src)

    def load_kv_masks(lo, hi, half=0):
        # interleave mask runs and valp runs in first-need order
        evs = []
        for (c0, k) in MASK_GROUPS:
            if lo <= c0 < hi:
                evs.append((c0, 'm', (c0, k)))
        if half == 0:
            for (c0, k) in VALP_RUNS:
                if lo <= c0 < hi:
                    evs.append((c0 + 4, 'v', (c0, k)))  # needed ~LAG later
        evs.sort()
        for _, kind, (c0, k) in evs:
            if kind == 'm':
                _mask_half_dma(c0, k, half)
            else:
                _grouped_dma(valp_run_t[(c0, k)], valp, c0, k, H * 65)

    # raw K tiles double as the score stationary operand; q projections
    qp2 = [kqp.tile([128, QLEN], F16, tag=f"qp2_{p}", name=f"qp2_{p}")
           for p in range(4)]
    concatT = [const.tile([128, QLEN], F16, tag=f"ct{p}", name=f"ct{p}")
               for p in range(4)]

    xs = {}

    def proj_load(pair):
        # HWDGE on the ACT queue: no Pool desc-gen cost, and deferred call
        # sites keep these transfers out of the resident-stream window.
        # xq first: the q-projection chain gates the first scores.
        xq = xst.tile([128, QLEN], F16, tag="xq", name=f"xq{pair}")
        nc.scalar.dma_start(xq, xqT[pair * 128:(pair + 1) * 128, :])
        xk0 = xst.tile([128, S // 2], F16, tag="xka", name=f"xka{pair}")
        nc.scalar.dma_start(xk0, xkT[pair * 128:(pair + 1) * 128, 0:S // 2])
        xk1 = xst.tile([128, S // 2], F16, tag="xkb", name=f"xkb{pair}")
        nc.scalar.dma_start(xk1, xkT[pair * 128:(pair + 1) * 128, S // 2:])
        xs[pair] = ((xk0, xk1), xq)

    def make_proj_chunk(pair, c):
        def run():
            _, xq = xs[pair]
            ps = psb.tile([128, 1024], F32, tag="ps", name=f"q{pair}_{c}")
            nc.tensor.matmul(ps[:, 0:512], lhsT=g2_sb,
                             rhs=xq[:, c * 512:(c + 1) * 512],
                             start=True, stop=True)
            nc.scalar.copy(qp2[pair][:, c * 512:(c + 1) * 512], ps[:, 0:512])
        return run

    # act-chunk mask engine alternation
    msk_i = [0]

    def msk_engine():
        e = MSK_CYCLE[msk_i[0] % len(MSK_CYCLE)]
        msk_i[0] += 1
        return nc.gpsimd if e == 'p' else nc.vector

    # ------------- flat software-pipelined attention stream -------------
    # 8 blocks x 64 half-units, one stream: scores(u) || exp(u) ||
    # AV(u-LAG) || deferred tail/transpose/proj/outproj works. AV matmuls of
    # block N drain while block N+1's scores stream, so PE never idles at
    # block boundaries; tail work is emitted a few units late so it never
    # parks at the head of an in-order engine queue.
    BLOCKS = [(p, qw) for p in range(4) for qw in range(2)]
    LAG = 10

    import heapq
    from collections import deque
    due = []         # heap of (due_u, seq, fn)
    seq_i = [0]
    u_now = [0]

    def sched(du, fn):
        heapq.heappush(due, (du, seq_i[0], fn))
        seq_i[0] += 1

    def pop_due(limit=2):
        n = 0
        while due and due[0][0] <= u_now[0] and n < limit:
            _, _, fn = heapq.heappop(due)
            fn()
            n += 1

    def scores_half(pair, qw, kc, h2, ps):
        """One [128,512] score matmul into bank h2 of the fused kc tile;
        'pe' chunks also fold the mask in via an fp8 DoubleRow identity."""
        (xk0, xk1), _ = xs[pair]
        xk = xk0 if kc < KC // 2 else xk1
        koff = 0 if kc < KC // 2 else S // 2
        dst = ps[:, h2 * 512:(h2 + 1) * 512]
        pe_cls = CLS[kc] == 'pe'
        nc.tensor.matmul(dst, lhsT=xk[h2 * 64:(h2 + 1) * 64,
                                      kc * 128 - koff:(kc + 1) * 128 - koff],
                         rhs=qp2[pair][h2 * 64:(h2 + 1) * 64,
                                       qw * 512:(qw + 1) * 512],
                         start=True, stop=not pe_cls)
        if pe_cls:
            # += 10 * (m*24) = 240*m into the bank (0.5 cycles/row)
            i2v = bass.AP(tensor=i2.tensor, offset=i2.offset,
                          ap=[i2.ap[0], [128, 2], [1, 128]])
            ms = mbx_res[kc][:, qw * 512:(qw + 1) * 512]
            mv = bass.AP(tensor=ms.tensor, offset=ms.offset,
                         ap=[ms.ap[0], [0, 2], [1, 512]])
            nc.tensor.matmul(dst, lhsT=i2v, rhs=mv, start=False, stop=True,
                             perf_mode=mybir.MatmulPerfMode.DoubleRow)

    def _h2view(t, half_elems=512):
        return bass.AP(tensor=t.tensor, offset=t.offset,
                       ap=[t.ap[0], [half_elems, 2], [1, half_elems]])

    def expmask_fused(pair, qw, kc, ps):
        """One elementwise op over both heads' banks [128,1024]; the mask
        slice broadcasts across h2 via a stride-0 middle dim."""
        ms = mbx_res[kc][:, qw * 512:(qw + 1) * 512]
        ms2 = bass.AP(tensor=ms.tensor, offset=ms.offset,
                      ap=[ms.ap[0], [0, 2], [1, 512]])
        if CLS[kc] == 'saw':
            pti = ptp.tile([128, 1024], I16, tag="pt",
                           name=f"pt{pair}_{qw}_{kc}")
            nc.vector.scalar_tensor_tensor(_h2view(pti), _h2view(ps), BB8,
                                           ms2, ALU.add, ALU.mult)
            return pti.bitcast(BF16)
        pt = ptp.tile([128, 1024], BF16, tag="pt", name=f"pt{pair}_{qw}_{kc}")
        bias = biasC if CLS[kc] == 'pe' else biasB
        nc.scalar.activation(pt, ps, AF.Exp, bias=bias, scale=EXPSCALE)
        if CLS[kc] == 'act':
            msk_engine().tensor_mul(_h2view(pt), _h2view(pt), ms2)
        return pt

    cx_t = {}        # (bi, h2) -> cx tile [128, 512] (one PSUM bank)
    ctn_t = {}       # (bi, h2, qt) -> normalized ctx tile

    def emit_av(bi, kc, h2, pt):
        pair, qw = BLOCKS[bi]
        cx = cx_t.get((bi, h2))
        if cx is None:
            cx = ctxp.tile([128, 512], F32, tag="cx", name=f"cx{bi}_{h2}")
            cx_t[(bi, h2)] = cx
        h = 2 * pair + h2
        # qt blocks at qt*65 share one 2KB psum bank; only the first matmul
        # sets start (its pending-zero covers the whole bank).
        for qt in range(4):
            nc.tensor.matmul(
                cx[:, qt * 65:qt * 65 + 65],
                lhsT=pt[:, qt * 128:qt * 128 + 128],
                rhs=valp_t[kc][:, h * 65:(h + 1) * 65],
                start=(kc == 0 and qt == 0), stop=(kc == KC - 1),
                skip_group_check=True)
        if kc == KC - 1:
            sched(u_now[0] + 2, make_tail(bi, h2))

    def make_tail(bi, h2):
        pair, qw = BLOCKS[bi]

        def tail():
            cx = cx_t[(bi, h2)]
            r = ctn_p.tile([128, 4], F32, tag="rec", name=f"rc{bi}_{h2}")
            dn = bass.AP(tensor=cx.tensor, offset=cx.offset + 64,
                         ap=[cx.ap[0], [65, 4]])
            with nc.allow_low_precision(reason="softmax denom reciprocal"):
                nc.vector.reciprocal(r, dn)
            # one fused normalize: (cx qt-blocks) * (r broadcast per qt).
            # Both h2 write one shared [128,512] tile with col layout
            # (qt*128 + h2*64 + d) so each qt slab is a PLAIN [128,128]
            # full-partition DMA transpose straight into concatT.
            if h2 == 0:
                t = ctn_p.tile([128, 512], F16, tag="ctn", name=f"cn{bi}")
                ctn_t[bi] = t
            t = ctn_t[bi]
            t3 = bass.AP(tensor=t.tensor, offset=t.offset + h2 * 64,
                         ap=[t.ap[0], [128, 4], [1, 64]])
            cx3 = bass.AP(tensor=cx.tensor, offset=cx.offset,
                          ap=[cx.ap[0], [65, 4], [1, 64]])
            r3 = bass.AP(tensor=r.tensor, offset=r.offset,
                         ap=[r.ap[0], [1, 4], [0, 64]])
            nc.vector.tensor_mul(t3, cx3, r3)
            if h2 == 1:
                late = bi == 7
                for qt in range(4):
                    sched(u_now[0] + (1 + qt if late else 3 + 2 * qt),
                          make_transp(bi, qt))
                    if late:
                        sched(u_now[0] + 3 + qt, make_outproj(4 + qt))
        return tail

    def make_transp(bi, qt):
        pair, qw = BLOCKS[bi]

        def go():
            # plain [128,128] xbar transpose: ctn qt-slab (cols h2*64+d)
            # -> concatT rows (h2*64+d), cols (qw*512 + qt*128 + q)
            src = ctn_t[bi][:, qt * 128:qt * 128 + 128]
            dst = concatT[pair][:, qw * 512 + qt * 128:qw * 512 + qt * 128 + 128]
            nc.sync.dma_start_transpose(dst, src)
        return go

    def make_outproj(qt):
        def go():
            opb = psb.tile([128, 1024], F32, tag="ps", name=f"op{qt}")
            op = opb[:, 0:512]
            for pc in range(4):
                nc.tensor.matmul(op,
                                 lhsT=concatT[pc][:, qt * 128:(qt + 1) * 128],
                                 rhs=woe_sb[pc],
                                 start=(pc == 0), stop=(pc == 3))
            osb = osb_p.tile([128, E], F32, tag="osb", name=f"osb{qt}")
            nc.vector.scalar_tensor_tensor(osb, op, 1.0, bo_sb,
                                           ALU.mult, ALU.add)
            nc.sync.dma_start(out[qt * 128:(qt + 1) * 128, :], osb)
        return go

    # ---------------- schedule ----------------
    proj_load(0)
    load_kv_masks(0, 8, half=0)
    make_proj_chunk(0, 0)()
    make_proj_chunk(0, 1)()
    load_kv_masks(8, KC, half=0)
    load_kv_masks(0, KC, half=1)
    load_late_consts()

    # deferred proj loads/chunks: pair p's x loads land mid-block (after the
    # resident-stream window); its q-proj chunks a block before first use.
    sched(40, lambda: proj_load(1))
    sched(72, make_proj_chunk(1, 0))
    sched(76, make_proj_chunk(1, 1))
    sched(112, lambda: proj_load(2))
    sched(136, make_proj_chunk(2, 0))
    sched(140, make_proj_chunk(2, 1))
    sched(176, lambda: proj_load(3))
    sched(200, make_proj_chunk(3, 0))
    sched(204, make_proj_chunk(3, 1))
    # out-proj for q rows 0-511 after block 6's transposes; rest at drain.
    for i, qt in enumerate(range(4)):
        sched(482 + 4 * i, make_outproj(qt))

    pend = deque()
    for bi, (pair, qw) in enumerate(BLOCKS):
        for kc in range(KC):
            ps = psb.tile([128, 1024], F32, tag="ps",
                          name=f"ps{pair}_{qw}_{kc}")
            for h2 in range(2):
                scores_half(pair, qw, kc, h2, ps)
                if len(pend) >= LAG:
                    emit_av(*pend.popleft())
                    # fast-drain the previous block's trailing AVs so its
                    # tail (and the cx bank) frees before this block's
                    # first AV needs the ctxp slot
                    if pend and pend[0][0] != bi and len(pend) >= LAG - 4:
                        emit_av(*pend.popleft())
                if h2 == 1:
                    pt = expmask_fused(pair, qw, kc, ps)
                    pend.append((bi, kc, 0, pt[:, 0:512]))
                    pend.append((bi, kc, 1, pt[:, 512:1024]))
                pop_due()
                u_now[0] += 1
    while pend:
        emit_av(*pend.popleft())
        pop_due()
        u_now[0] += 1
    # flush remaining deferred work (last tails, transposes, out-proj 4-7)
    while due:
        pop_due(limit=2)
        u_now[0] += 1

    ctx.close()


def _prep_inputs(key, query, value, mask, Wq, Wk, Wv, Wo, bo):
    f16 = np.float16
    bf16 = ml_dtypes.bfloat16
    key = np.asarray(key, np.float32)
    query = np.asarray(query, np.float32)
    value = np.asarray(value, np.float32)
    mask = np.asarray(mask)
    Wq = np.asarray(Wq, np.float32)
    Wk = np.asarray(Wk, np.float32)
    Wv = np.asarray(Wv, np.float32)
    Wo = np.asarray(Wo, np.float32)
    bo = np.asarray(bo, np.float32)

    # Wo_eff[e, h*64+u] = sum_d Wo[e, h*64+d] * Wv[d, u]
    wo_eff = np.empty((E, E), np.float32)
    for h in range(H):
        wo_eff[:, h * DH:(h + 1) * DH] = Wo[:, h * DH:(h + 1) * DH] @ Wv
    woe = np.ascontiguousarray(wo_eff.T).astype(f16)   # [(h,u), e]

    # G = Wq^T @ Wk folded q-side projection (scaled so saw-chunk masks are
    # fp8-exact m*176); blockdiag over the head pair
    G = (Wq.T @ Wk).astype(np.float32) * np.float32(QSCALE)
    g2 = np.zeros((128, 128), np.float32)
    g2[0:64, 0:64] = G
    g2[64:128, 64:128] = G

    m01 = (mask[0, 0] != 0).astype(np.float32).T  # [k, q] in {0,1}

    fp8 = ml_dtypes.float8_e4m3
    i2 = np.zeros((128, 256), np.float32)
    i2[:, 0:128] = np.eye(128) * 10.0
    common = {
        "g2": g2.astype(f16),
        "woe": woe,
        "bo_b": np.ascontiguousarray(np.broadcast_to(bo, (128, E))).astype(np.float32),
        "eye": np.eye(128, dtype=np.float32).astype(f16),
        "i2": i2.astype(fp8),
    }
    per_b = {}
    for b in range(B):
        vp = np.ones((S, H, 65), np.float32)
        vp[:, :, :64] = value[b].reshape(S, H, DH)
        per_b[b] = {
            "xkT": np.ascontiguousarray(key[b].T).astype(f16),
            "valp": np.ascontiguousarray(vp.reshape(S, H * 65)).astype(bf16),
            "qT": query[b].T,
        }
    fp8 = ml_dtypes.float8_e4m3
    in_maps = []
    for c in range(N_CORES):
        b, qs = c // 4, (c % 4) * QLEN
        msl = np.ascontiguousarray(m01[:, qs:qs + QLEN])
        in_maps.append({
            "xkT": per_b[b]["xkT"],
            "xqT": np.ascontiguousarray(per_b[b]["qT"][:, qs:qs + QLEN]).astype(f16),
            "valp": per_b[b]["valp"],
            "mbx": msl.astype(bf16),
            "mb8": (msl * np.float32(V8)).astype(fp8),
            **common,
        })
    return in_maps


def get_module():
    if "nc" not in _CACHE:
        _CACHE["nc"] = _build_module()
    return _CACHE["nc"]


def kernel(key, query, value, mask, Wq, Wk, Wv, Wo, bo, **_):
    nc = get_module()
    in_maps = _prep_inputs(key, query, value, mask, Wq, Wk, Wv, Wo, bo)
    res = bass_utils.run_bass_kernel_spmd(
        nc, in_maps, core_ids=list(range(N_CORES)))
    full = np.empty((B, S, E), np.float32)
    for c in range(N_CORES):
        b, qs = c // 4, (c % 4) * QLEN
        full[b, qs:qs + QLEN, :] = res.results[c]["out"]
    return full


# revision 76
# speedup vs baseline: 1.0598x; 1.0598x over previous
"""Trainium2 Bass kernel for 8-head MultiHeadAttention (B=2, S=4096, E=512).

Sharding: 8 cores = 2 batches x 4 query-row chunks of 1024; each core runs
all 8 heads for its (batch, q-range) as 4 head-pairs x 2 query-windows, with
the k-dimension processed in 32 chunks of 128.

v2 design (vs v0):
- K-projection is folded into Q on the host: G = Wq^T @ Wk, so
  scores = (x_q G) . x_k^T and the raw (f16) K tiles are the score matmul's
  stationary operand directly. Only a tiny Q-side projection remains
  (1 blockdiag matmul per 512-col chunk). Wv stays folded into Wo.
- Whole 16-bit pipeline is f16 (x, qp, pt, V, ctx, Wo) - f16's 10-bit
  mantissa keeps the base quantization error ~6e-4, leaving the error
  budget to the Schraudolph trick.
- exp via f16-bits Schraudolph on DVE *and* Pool (both support
  scalar_tensor_tensor): ONE op computes i16 = round((s + 82.93) *
  (184.5 * mask)) whose bits ARE the f16 weights; mask folded in as the
  multiplicand. 22 of 32 k-chunks go this way; 10 use the ACT table exp
  (f16 out) with a post-exp f16 mask multiply on Pool/DVE.
- PE only does matmuls: scores (f16, 512 rows), attention-value flip
  (pt stationary, V+ones moving, 65 rows), q-proj, transposes, out-proj.
  No mask work on PE.
- cx accumulators bank-aligned in PSUM; AV runs 8 half-units behind
  scores; transposes/out-proj trickle into later attention blocks.
"""
import sys
for _p in ('/root/.axon_site/_ro/trn_rl_repo', '/opt/trn_rl_repo'):
    if _p not in sys.path:
        sys.path.append(_p)

import numpy as np
import ml_dtypes

import concourse.bass as bass
import concourse.tile as tile
from concourse import bacc, mybir
from concourse import bass_utils

F32 = mybir.dt.float32
F16 = mybir.dt.float16
BF16 = mybir.dt.bfloat16
FP8 = mybir.dt.float8e4
I16 = mybir.dt.int16
AF = mybir.ActivationFunctionType
ALU = mybir.AluOpType

N_CORES = 8
B, S, E, H, DH = 2, 4096, 512, 8, 64
QLEN = S // 4           # 1024 q rows per core
KC = S // 128           # 32 k chunks

# f16-bits Schraudolph: i16 = round((s + BB) * (A16 * mask)); the i16 bit
# pattern read as f16 is ~exp(s/8). A16 = 1024*log2(e)/8 rounded to an
# f16-exact value; BB calibrated so the mean weight ratio vs exact exp is 1.
# The saw-chunk mask is stored fp8 as m*176 (e4m3-exact); the 184.5/176
# ratio is folded into G on the host (scores come out scaled by 184.5/176),
# so bits = (s' + BB8) * (176 * m) == (s + BB) * (184.5 * m) exactly.
# bf16-bits variant: the dataset's extreme scores (|s| up to ~92, heavy
# product-normal tails) give weights up to e^11.4 = 9e4 > f16 max, so the
# weights (pt) live in bf16. bits = (QSCALE*s + BB8) * (24 * m): 24 is
# e4m3-exact, QSCALE = (128*log2(e)/8)/24 is folded into G on the host,
# BB8 calibrated for mean weight ratio 1 (rms sawtooth ~1.78%).
ABITS = 23.083120654223414      # 128*log2(e)/8
V8 = 24.0                       # fp8 mask multiplier
QSCALE = ABITS / V8             # host folds this into G
BB8 = 677.026428
EXPSCALE = 0.125 / QSCALE       # ACT exp scale on the rescaled scores

# per-kc class: 'saw' = Schraudolph stt (mask folded, on DVE or Pool);
# 'act' = ACT table exp (f16) + post-exp mask multiply on DVE.
# per-kc classes (GPSIMD cannot touch PSUM, so exp lives on ACT+DVE only):
#  'saw': DVE Schraudolph stt, mask folded (fp8 m*176 operand)
#  'act': ACT table exp + post-exp f16 mask multiply (DVE or Pool, SBUF)
#  'pe':  ACT table exp, mask pre-added into the score PSUM by an fp8
#         DoubleRow identity matmul (i2=eye*1.5 against the m*176 fp8 mask
#         adds 264*m; exp bias -264*EXPSCALE turns masked scores into ~e-31)
# period-8 layout keeps runs short for engine smoothness and mask DMAs
# groupable by dtype run (mb8 for saw+pe, f16 for act).
# act = all odd kcs (ACT-exp runs stay <= 3 incl. the pe islands); pe on 4
# spread evens; saw on the other 12 evens. Odd/even interleave keeps every
# engine's per-unit load smooth AND leaves each dtype's chunks on an
# affine stride-2 grid so mask DMAs still group (4 chunks per transfer).
ACT_KCS = set(range(1, KC, 2))
PE_KCS = {4, 12, 20, 28}
CLS = {kc: ('act' if kc % 2 else ('pe' if kc in PE_KCS else 'saw'))
       for kc in range(KC)}
# (c0, n_chunks) with chunk stride 2; dtype = f16 for odd c0, fp8 for even
MASK_GROUPS = [(0, 4), (1, 4), (8, 4), (9, 4), (16, 4), (17, 4),
               (24, 4), (25, 4)]
VALP_RUNS = [(0, 1), (1, 1), (2, 2), (4, 2), (6, 2), (8, 4), (12, 4),
             (16, 4), (20, 4), (24, 4), (28, 4)]

# act-chunk mask multiplies alternate DVE / Pool (both SBUF-legal; Pool
# runs TT at 0.42x roofline so it only takes ~60%)
MSK_CYCLE = ('d', 'd', 'p', 'd', 'd', 'p', 'd', 'p')

_CACHE = {}


def _build_module():
    nc = bacc.Bacc("TRN2", target_bir_lowering=False, debug=False,
                   enable_asserts=True, num_devices=N_CORES)

    xkT = nc.dram_tensor("xkT", [E, S], F16, kind="ExternalInput").ap()
    xqT = nc.dram_tensor("xqT", [E, QLEN], F16, kind="ExternalInput").ap()
    valp = nc.dram_tensor("valp", [S, H * 65], BF16, kind="ExternalInput").ap()
    mbx = nc.dram_tensor("mbx", [S, QLEN], BF16, kind="ExternalInput").ap()
    mb8 = nc.dram_tensor("mb8", [S, QLEN], FP8, kind="ExternalInput").ap()
    g2 = nc.dram_tensor("g2", [128, 128], F16, kind="ExternalInput").ap()
    woe = nc.dram_tensor("woe", [E, E], F16, kind="ExternalInput").ap()
    bo_b = nc.dram_tensor("bo_b", [128, E], F32, kind="ExternalInput").ap()
    eye_d = nc.dram_tensor("eye", [128, 128], F16, kind="ExternalInput").ap()
    i2_d = nc.dram_tensor("i2", [128, 256], FP8, kind="ExternalInput").ap()
    out = nc.dram_tensor("out", [QLEN, E], F32, kind="ExternalOutput").ap()

    with tile.TileContext(nc) as tc:
        _emit(tc, nc, xkT, xqT, valp, mbx, mb8, g2, woe, bo_b, eye_d, i2_d,
              out)

    nc.compile()
    return nc


def _emit(tc, nc, xkT, xqT, valp, mbx, mb8, g2, woe, bo_b, eye_d, i2_d, out):
    from contextlib import ExitStack
    ctx = ExitStack()
    const = ctx.enter_context(tc.tile_pool(name="const", bufs=1))
    kqp = ctx.enter_context(tc.tile_pool(name="kqp", bufs=1))
    xst = ctx.enter_context(tc.tile_pool(name="xst", bufs=2))
    ptp = ctx.enter_context(tc.tile_pool(name="pt", bufs=10))
    ctn_p = ctx.enter_context(tc.tile_pool(name="ctn", bufs=5))
    osb_p = ctx.enter_context(tc.tile_pool(name="osb", bufs=4))
    psb = ctx.enter_context(tc.tile_pool(name="psb", bufs=3, space="PSUM"))
    ctxp = ctx.enter_context(tc.tile_pool(name="ctxp", bufs=2, space="PSUM"))

    # ---------------- constants ----------------
    g2_sb = const.tile([128, 128], F16, tag="g2")
    nc.sync.dma_start(g2_sb, g2)
    eye = const.tile([128, 128], F16, tag="eye")
    nc.sync.dma_start(eye, eye_d)
    i2 = const.tile([128, 256], FP8, tag="i2")
    nc.sync.dma_start(i2, i2_d)
    woe_sb = [const.tile([128, E], F16, tag=f"woe{pc}", name=f"woe{pc}")
              for pc in range(4)]
    bo_sb = const.tile([128, E], F32, tag="bo")
    biasB = const.tile([128, 1], F32, tag="biasB")
    nc.vector.memset(biasB, 0.0)
    biasC = const.tile([128, 1], F32, tag="biasC")
    nc.vector.memset(biasC, -240.0 * EXPSCALE)

    def load_late_consts():
        for pc in range(4):
            nc.sync.dma_start(woe_sb[pc], woe[pc * 128:(pc + 1) * 128, :])
        nc.sync.dma_start(bo_sb, bo_b)

    # resident masks + V: one run-tile per contiguous same-class chunk run,
    # loaded by ONE grouped DMA each (HWDGE desc-gen would otherwise
    # serialize 64 transfers at ~600ns apiece). saw-chunk masks are fp8
    # (m*176); act-chunk masks f16 (keeps the DVE TT in 2x mode).
    mbx_res = {}
    mask_run_t = {}
    for (c0, k) in MASK_GROUPS:
        dt = FP8 if CLS[c0] != 'act' else BF16
        rt = const.tile([128, k * QLEN], dt, tag=f"mr{c0}", name=f"mr{c0}")
        mask_run_t[(c0, k)] = rt
        for j in range(k):
            mbx_res[c0 + 2 * j] = rt[:, j * QLEN:(j + 1) * QLEN]
    valp_run_t = {}
    valp_t = [None] * KC
    for (c0, k) in VALP_RUNS:
        rt = const.tile([128, k * H * 65], BF16, tag=f"vr{c0}", name=f"vr{c0}")
        valp_run_t[(c0, k)] = rt
        for j in range(k):
            valp_t[c0 + j] = rt[:, j * H * 65:(j + 1) * H * 65]

    def _grouped_dma(dst, src_ap, c0, k, row_elems):
        src = bass.AP(tensor=src_ap.tensor,
                      offset=src_ap.offset + c0 * 128 * row_elems,
                      ap=[[row_elems, 128], [128 * row_elems, k],
                          [1, row_elems]])
        nc.sync.dma_start(dst, src)

    def _mask_half_dma(c0, k, half):
        # load q-columns [half*512, half*512+512) of k stride-2 chunks in
        # one DMA: block 0 only touches the qw=0 halves, so splitting keeps
        # the critical first-block stream under its consumption rate.
        rt = mask_run_t[(c0, k)]
        src_ap = mb8 if CLS[c0] != 'act' else mbx
        dst = bass.AP(tensor=rt.tensor, offset=rt.offset + half * 512,
                      ap=[rt.ap[0], [QLEN, k], [1, 512]])
        src = bass.AP(tensor=src_ap.tensor,
                      offset=src_ap.offset + c0 * 128 * QLEN + half * 512,
                      ap=[[QLEN, 128], [2 * 128 * QLEN, k], [1, 512]])
        nc.sync.dma_start(dst, src)

    def load_kv_masks(lo, hi, half=0):
        # interleave mask runs and valp runs in first-need order
        evs = []
        for (c0, k) in MASK_GROUPS:
            if lo <= c0 < hi:
                evs.append((c0, 'm', (c0, k)))
        if half == 0:
            for (c0, k) in VALP_RUNS:
                if lo <= c0 < hi:
                    evs.append((c0 + 4, 'v', (c0, k)))  # needed ~LAG later
        evs.sort()
        for _, kind, (c0, k) in evs:
            if kind == 'm':
                _mask_half_dma(c0, k, half)
            else:
                _grouped_dma(valp_run_t[(c0, k)], valp, c0, k, H * 65)

    # raw K tiles double as the score stationary operand; q projections
    qp2 = [kqp.tile([128, QLEN], F16, tag=f"qp2_{p}", name=f"qp2_{p}")
           for p in range(4)]
    concatT = [const.tile([128, QLEN], F16, tag=f"ct{p}", name=f"ct{p}")
               for p in range(4)]

    xs = {}

    def proj_load(pair):
        # HWDGE on the ACT queue: no Pool desc-gen cost, and deferred call
        # sites keep these transfers out of the resident-stream window.
        # xq first: the q-projection chain gates the first scores.
        xq = xst.tile([128, QLEN], F16, tag="xq", name=f"xq{pair}")
        nc.scalar.dma_start(xq, xqT[pair * 128:(pair + 1) * 128, :])
        xk0 = xst.tile([128, S // 2], F16, tag="xka", name=f"xka{pair}")
        nc.scalar.dma_start(xk0, xkT[pair * 128:(pair + 1) * 128, 0:S // 2])
        xk1 = xst.tile([128, S // 2], F16, tag="xkb", name=f"xkb{pair}")
        nc.scalar.dma_start(xk1, xkT[pair * 128:(pair + 1) * 128, S // 2:])
        xs[pair] = ((xk0, xk1), xq)

    def make_proj_chunk(pair, c):
        def run():
            _, xq = xs[pair]
            ps = psb.tile([128, 1024], F32, tag="ps", name=f"q{pair}_{c}")
            nc.tensor.matmul(ps[:, 0:512], lhsT=g2_sb,
                             rhs=xq[:, c * 512:(c + 1) * 512],
                             start=True, stop=True)
            nc.scalar.copy(qp2[pair][:, c * 512:(c + 1) * 512], ps[:, 0:512])
        return run

    # act-chunk mask engine alternation
    msk_i = [0]

    def msk_engine():
        e = MSK_CYCLE[msk_i[0] % len(MSK_CYCLE)]
        msk_i[0] += 1
        return nc.gpsimd if e == 'p' else nc.vector

    # ------------- flat software-pipelined attention stream -------------
    # 8 blocks x 64 half-units, one stream: scores(u) || exp(u) ||
    # AV(u-LAG) || deferred tail/transpose/proj/outproj works. AV matmuls of
    # block N drain while block N+1's scores stream, so PE never idles at
    # block boundaries; tail work is emitted a few units late so it never
    # parks at the head of an in-order engine queue.
    BLOCKS = [(p, qw) for p in range(4) for qw in range(2)]
    LAG = 14

    import heapq
    from collections import deque
    due = []         # heap of (due_u, seq, fn)
    seq_i = [0]
    u_now = [0]

    def sched(du, fn):
        heapq.heappush(due, (du, seq_i[0], fn))
        seq_i[0] += 1

    def pop_due(limit=2):
        n = 0
        while due and due[0][0] <= u_now[0] and n < limit:
            _, _, fn = heapq.heappop(due)
            fn()
            n += 1

    def scores_half(pair, qw, kc, h2, ps):
        """One [128,512] score matmul into bank h2 of the fused kc tile;
        'pe' chunks also fold the mask in via an fp8 DoubleRow identity."""
        (xk0, xk1), _ = xs[pair]
        xk = xk0 if kc < KC // 2 else xk1
        koff = 0 if kc < KC // 2 else S // 2
        dst = ps[:, h2 * 512:(h2 + 1) * 512]
        pe_cls = CLS[kc] == 'pe'
        nc.tensor.matmul(dst, lhsT=xk[h2 * 64:(h2 + 1) * 64,
                                      kc * 128 - koff:(kc + 1) * 128 - koff],
                         rhs=qp2[pair][h2 * 64:(h2 + 1) * 64,
                                       qw * 512:(qw + 1) * 512],
                         start=True, stop=not pe_cls)
        if pe_cls:
            # += 10 * (m*24) = 240*m into the bank (0.5 cycles/row)
            i2v = bass.AP(tensor=i2.tensor, offset=i2.offset,
                          ap=[i2.ap[0], [128, 2], [1, 128]])
            ms = mbx_res[kc][:, qw * 512:(qw + 1) * 512]
            mv = bass.AP(tensor=ms.tensor, offset=ms.offset,
                         ap=[ms.ap[0], [0, 2], [1, 512]])
            nc.tensor.matmul(dst, lhsT=i2v, rhs=mv, start=False, stop=True,
                             perf_mode=mybir.MatmulPerfMode.DoubleRow)

    def _h2view(t, half_elems=512):
        return bass.AP(tensor=t.tensor, offset=t.offset,
                       ap=[t.ap[0], [half_elems, 2], [1, half_elems]])

    def expmask_fused(pair, qw, kc, ps):
        """One elementwise op over both heads' banks [128,1024]; the mask
        slice broadcasts across h2 via a stride-0 middle dim."""
        ms = mbx_res[kc][:, qw * 512:(qw + 1) * 512]
        ms2 = bass.AP(tensor=ms.tensor, offset=ms.offset,
                      ap=[ms.ap[0], [0, 2], [1, 512]])
        if CLS[kc] == 'saw':
            pti = ptp.tile([128, 1024], I16, tag="pt",
                           name=f"pt{pair}_{qw}_{kc}")
            nc.vector.scalar_tensor_tensor(_h2view(pti), _h2view(ps), BB8,
                                           ms2, ALU.add, ALU.mult)
            return pti.bitcast(BF16)
        pt = ptp.tile([128, 1024], BF16, tag="pt", name=f"pt{pair}_{qw}_{kc}")
        bias = biasC if CLS[kc] == 'pe' else biasB
        nc.scalar.activation(pt, ps, AF.Exp, bias=bias, scale=EXPSCALE)
        if CLS[kc] == 'act':
            e = MSK_CYCLE[msk_i[0] % len(MSK_CYCLE)]
            msk_i[0] += 1
            if e == 'd':
                nc.vector.tensor_mul(_h2view(pt), _h2view(pt), ms2)
            else:
                # split: DVE takes bank A, Pool bank B (Pool TT is 0.42x
                # roofline; half-tiles keep it off the pt critical path)
                nc.vector.tensor_mul(pt[:, 0:512], pt[:, 0:512], ms)
                nc.gpsimd.tensor_mul(pt[:, 512:1024], pt[:, 512:1024], ms)
        return pt

    cx_t = {}        # (bi, h2) -> cx tile [128, 512] (one PSUM bank)
    ctn_t = {}       # (bi, h2, qt) -> normalized ctx tile

    def emit_av(bi, kc, h2, pt):
        pair, qw = BLOCKS[bi]
        cx = cx_t.get((bi, h2))
        if cx is None:
            cx = ctxp.tile([128, 512], F32, tag="cx", name=f"cx{bi}_{h2}")
            cx_t[(bi, h2)] = cx
        h = 2 * pair + h2
        # qt blocks at qt*65 share one 2KB psum bank; only the first matmul
        # sets start (its pending-zero covers the whole bank).
        for qt in range(4):
            nc.tensor.matmul(
                cx[:, qt * 65:qt * 65 + 65],
                lhsT=pt[:, qt * 128:qt * 128 + 128],
                rhs=valp_t[kc][:, h * 65:(h + 1) * 65],
                start=(kc == 0 and qt == 0), stop=(kc == KC - 1),
                skip_group_check=True)
        if kc == KC - 1:
            sched(u_now[0] + 2, make_tail(bi, h2))

    def make_tail(bi, h2):
        pair, qw = BLOCKS[bi]

        def tail():
            cx = cx_t[(bi, h2)]
            r = ctn_p.tile([128, 4], F32, tag="rec", name=f"rc{bi}_{h2}")
            dn = bass.AP(tensor=cx.tensor, offset=cx.offset + 64,
                         ap=[cx.ap[0], [65, 4]])
            with nc.allow_low_precision(reason="softmax denom reciprocal"):
                nc.vector.reciprocal(r, dn)
            # one fused normalize: (cx qt-blocks) * (r broadcast per qt).
            # Both h2 write one shared [128,512] tile with col layout
            # (qt*128 + h2*64 + d) so each qt slab is a PLAIN [128,128]
            # full-partition DMA transpose straight into concatT.
            if h2 == 0:
                t = ctn_p.tile([128, 512], F16, tag="ctn", name=f"cn{bi}")
                ctn_t[bi] = t
            t = ctn_t[bi]
            t3 = bass.AP(tensor=t.tensor, offset=t.offset + h2 * 64,
                         ap=[t.ap[0], [128, 4], [1, 64]])
            cx3 = bass.AP(tensor=cx.tensor, offset=cx.offset,
                          ap=[cx.ap[0], [65, 4], [1, 64]])
            r3 = bass.AP(tensor=r.tensor, offset=r.offset,
                         ap=[r.ap[0], [1, 4], [0, 64]])
            nc.vector.tensor_mul(t3, cx3, r3)
            if h2 == 1:
                late = bi == 7
                for qt in range(4):
                    sched(u_now[0] + (1 + qt if late else 3 + 2 * qt),
                          make_transp(bi, qt))
                    if late:
                        sched(u_now[0] + 3 + qt, make_outproj(4 + qt))
        return tail

    def make_transp(bi, qt):
        pair, qw = BLOCKS[bi]

        def go():
            # plain [128,128] xbar transpose: ctn qt-slab (cols h2*64+d)
            # -> concatT rows (h2*64+d), cols (qw*512 + qt*128 + q)
            src = ctn_t[bi][:, qt * 128:qt * 128 + 128]
            dst = concatT[pair][:, qw * 512 + qt * 128:qw * 512 + qt * 128 + 128]
            nc.sync.dma_start_transpose(dst, src)
        return go

    def make_outproj(qt):
        def go():
            opb = psb.tile([128, 1024], F32, tag="ps", name=f"op{qt}")
            op = opb[:, 0:512]
            for pc in range(4):
                nc.tensor.matmul(op,
                                 lhsT=concatT[pc][:, qt * 128:(qt + 1) * 128],
                                 rhs=woe_sb[pc],
                                 start=(pc == 0), stop=(pc == 3))
            osb = osb_p.tile([128, E], F32, tag="osb", name=f"osb{qt}")
            nc.vector.scalar_tensor_tensor(osb, op, 1.0, bo_sb,
                                           ALU.mult, ALU.add)
            nc.sync.dma_start(out[qt * 128:(qt + 1) * 128, :], osb)
        return go

    # ---------------- schedule ----------------
    proj_load(0)
    load_kv_masks(0, 8, half=0)
    make_proj_chunk(0, 0)()
    make_proj_chunk(0, 1)()
    load_kv_masks(8, KC, half=0)
    load_kv_masks(0, KC, half=1)
    load_late_consts()

    # deferred proj loads/chunks: pair p's x loads land mid-block (after the
    # resident-stream window); its q-proj chunks a block before first use.
    sched(40, lambda: proj_load(1))
    sched(72, make_proj_chunk(1, 0))
    sched(76, make_proj_chunk(1, 1))
    sched(112, lambda: proj_load(2))
    sched(136, make_proj_chunk(2, 0))
    sched(140, make_proj_chunk(2, 1))
    sched(176, lambda: proj_load(3))
    sched(200, make_proj_chunk(3, 0))
    sched(204, make_proj_chunk(3, 1))
    # out-proj for q rows 0-511 after block 6's transposes; rest at drain.
    for i, qt in enumerate(range(4)):
        sched(482 + 4 * i, make_outproj(qt))

    pend = deque()
    for bi, (pair, qw) in enumerate(BLOCKS):
        for kc in range(KC):
            ps = psb.tile([128, 1024], F32, tag="ps",
                          name=f"ps{pair}_{qw}_{kc}")
            for h2 in range(2):
                scores_half(pair, qw, kc, h2, ps)
                if len(pend) >= LAG:
                    emit_av(*pend.popleft())
                    # fast-drain the previous block's trailing AVs so its
                    # tail (and the cx bank) frees before this block's
                    # first AV needs the ctxp slot
                    if pend and pend[0][0] != bi and len(pend) >= LAG - 4:
                        emit_av(*pend.popleft())
                if h2 == 1:
                    pt = expmask_fused(pair, qw, kc, ps)
                    pend.append((bi, kc, 0, pt[:, 0:512]))
                    pend.append((bi, kc, 1, pt[:, 512:1024]))
                pop_due()
                u_now[0] += 1
    while pend:
        emit_av(*pend.popleft())
        pop_due()
        u_now[0] += 1
    # flush remaining deferred work (last tails, transposes, out-proj 4-7)
    while due:
        pop_due(limit=2)
        u_now[0] += 1

    ctx.close()


def _prep_inputs(key, query, value, mask, Wq, Wk, Wv, Wo, bo):
    f16 = np.float16
    bf16 = ml_dtypes.bfloat16
    key = np.asarray(key, np.float32)
    query = np.asarray(query, np.float32)
    value = np.asarray(value, np.float32)
    mask = np.asarray(mask)
    Wq = np.asarray(Wq, np.float32)
    Wk = np.asarray(Wk, np.float32)
    Wv = np.asarray(Wv, np.float32)
    Wo = np.asarray(Wo, np.float32)
    bo = np.asarray(bo, np.float32)

    # Wo_eff[e, h*64+u] = sum_d Wo[e, h*64+d] * Wv[d, u]
    wo_eff = np.empty((E, E), np.float32)
    for h in range(H):
        wo_eff[:, h * DH:(h + 1) * DH] = Wo[:, h * DH:(h + 1) * DH] @ Wv
    woe = np.ascontiguousarray(wo_eff.T).astype(f16)   # [(h,u), e]

    # G = Wq^T @ Wk folded q-side projection (scaled so saw-chunk masks are
    # fp8-exact m*176); blockdiag over the head pair
    G = (Wq.T @ Wk).astype(np.float32) * np.float32(QSCALE)
    g2 = np.zeros((128, 128), np.float32)
    g2[0:64, 0:64] = G
    g2[64:128, 64:128] = G

    m01 = (mask[0, 0] != 0).astype(np.float32).T  # [k, q] in {0,1}

    fp8 = ml_dtypes.float8_e4m3
    i2 = np.zeros((128, 256), np.float32)
    i2[:, 0:128] = np.eye(128) * 10.0
    common = {
        "g2": g2.astype(f16),
        "woe": woe,
        "bo_b": np.ascontiguousarray(np.broadcast_to(bo, (128, E))).astype(np.float32),
        "eye": np.eye(128, dtype=np.float32).astype(f16),
        "i2": i2.astype(fp8),
    }
    per_b = {}
    for b in range(B):
        vp = np.ones((S, H, 65), np.float32)
        vp[:, :, :64] = value[b].reshape(S, H, DH)
        per_b[b] = {
            "xkT": np.ascontiguousarray(key[b].T).astype(f16),
            "valp": np.ascontiguousarray(vp.reshape(S, H * 65)).astype(bf16),
            "qT": query[b].T,
        }
    fp8 = ml_dtypes.float8_e4m3
    in_maps = []
    for c in range(N_CORES):
        b, qs = c // 4, (c % 4) * QLEN
        msl = np.ascontiguousarray(m01[:, qs:qs + QLEN])
        in_maps.append({
            "xkT": per_b[b]["xkT"],
            "xqT": np.ascontiguousarray(per_b[b]["qT"][:, qs:qs + QLEN]).astype(f16),
            "valp": per_b[b]["valp"],
            "mbx": msl.astype(bf16),
            "mb8": (msl * np.float32(V8)).astype(fp8),
            **common,
        })
    return in_maps


def get_module():
    if "nc" not in _CACHE:
        _CACHE["nc"] = _build_module()
    return _CACHE["nc"]


def kernel(key, query, value, mask, Wq, Wk, Wv, Wo, bo, **_):
    nc = get_module()
    in_maps = _prep_inputs(key, query, value, mask, Wq, Wk, Wv, Wo, bo)
    res = bass_utils.run_bass_kernel_spmd(
        nc, in_maps, core_ids=list(range(N_CORES)))
    full = np.empty((B, S, E), np.float32)
    for c in range(N_CORES):
        b, qs = c // 4, (c % 4) * QLEN
        full[b, qs:qs + QLEN, :] = res.results[c]["out"]
    return full


# revision 95
# speedup vs baseline: 1.0722x; 1.0117x over previous
"""Trainium2 Bass kernel for 8-head MultiHeadAttention (B=2, S=4096, E=512).

Sharding: 8 cores = 2 batches x 4 query-row chunks of 1024; each core runs
all 8 heads for its (batch, q-range) as 8 blocks (4 head-pairs x 2 query
windows) of 64 half-units, flattened into ONE software-pipelined stream.

Design:
- K-projection folded into Q on the host (G = Wq^T Wk, pre-scaled by
  QSCALE): scores = (x_q G) . x_k^T, so raw f16 K tiles are the score
  matmul's stationary operand and only a tiny Q-side projection remains.
  Wv is folded into Wo; the output bias rides as a K=1 ones-row matmul.
- Per kc, both heads' scores land in one [128,1024] two-bank PSUM tile;
  ONE fused elementwise op covers both: 'saw' chunks use a bf16-bits
  Schraudolph stt on DVE (i16 = round((s+BB8)*(24*m)) whose bits ARE the
  bf16 weights - mask folded free as the fp8 m*24 multiplicand); 'act'
  chunks use the ACT table exp with a post-exp bf16 mask multiply split
  DVE/Pool; 'pe' chunks use ACT exp with the mask pre-added into PSUM by
  an fp8 DoubleRow identity matmul (+240*m, exp bias -240*EXPSCALE).
  GPSIMD cannot touch PSUM, so Pool only gets SBUF-side mask work.
- Attention-value matmuls are flipped (pt stationary, V+ones moving, 65
  rows/matmul) and lag LAG half-units behind scores; AVs of block N drain
  while block N+1's scores stream, so PE never idles at block boundaries.
  The ones column lands the softmax denominator per partition.
- Tails (reciprocal + one broadcast-multiply normalize into a qt-major
  [128,512] tile) are deferred-scheduled a few units late so they never
  park at an in-order queue head; each qt slab then DMA-xbar-transposes
  straight into concatT (no PE transposes, no PSUM staging).
- Resident masks/V stream as grouped stride-2 multi-chunk DMAs (HWDGE
  desc-gen is ~600ns/DMA), split by q-half and V-columns by head-pair so
  block 0's critical window only loads what it reads.
"""
import sys
for _p in ('/root/.axon_site/_ro/trn_rl_repo', '/opt/trn_rl_repo'):
    if _p not in sys.path:
        sys.path.append(_p)

import numpy as np
import ml_dtypes

import concourse.bass as bass
import concourse.tile as tile
from concourse import bacc, mybir
from concourse import bass_utils

F32 = mybir.dt.float32
F16 = mybir.dt.float16
BF16 = mybir.dt.bfloat16
FP8 = mybir.dt.float8e4
I16 = mybir.dt.int16
AF = mybir.ActivationFunctionType
ALU = mybir.AluOpType

N_CORES = 8
B, S, E, H, DH = 2, 4096, 512, 8, 64
QLEN = S // 4           # 1024 q rows per core
KC = S // 128           # 32 k chunks

# bf16-bits Schraudolph: the dataset's extreme scores (|s| up to ~92, heavy
# product-normal tails) give weights up to e^11.4 = 9e4 > f16 max, so the
# weights (pt) live in bf16. bits = (QSCALE*s + BB8) * (24 * m): 24 is
# e4m3-exact, QSCALE = (128*log2(e)/8)/24 is folded into G on the host,
# BB8 calibrated for mean weight ratio 1 (rms sawtooth ~1.78%).
ABITS = 23.083120654223414      # 128*log2(e)/8
V8 = 24.0                       # fp8 mask multiplier
QSCALE = ABITS / V8             # host folds this into G
BB8 = 677.026428
EXPSCALE = 0.125 / QSCALE       # ACT exp scale on the rescaled scores

# per-kc classes (GPSIMD cannot touch PSUM, so exp lives on ACT+DVE only):
#  'saw': DVE Schraudolph stt, mask folded (fp8 m*176 operand)
#  'act': ACT table exp + post-exp f16 mask multiply (DVE or Pool, SBUF)
#  'pe':  ACT table exp, mask pre-added into the score PSUM by an fp8
#         DoubleRow identity matmul (i2=eye*1.5 against the m*176 fp8 mask
#         adds 264*m; exp bias -264*EXPSCALE turns masked scores into ~e-31)
# period-8 layout keeps runs short for engine smoothness and mask DMAs
# groupable by dtype run (mb8 for saw+pe, f16 for act).
# act = all odd kcs (ACT-exp runs stay <= 3 incl. the pe islands); pe on 4
# spread evens; saw on the other 12 evens. Odd/even interleave keeps every
# engine's per-unit load smooth AND leaves each dtype's chunks on an
# affine stride-2 grid so mask DMAs still group (4 chunks per transfer).
ACT_KCS = set(range(1, KC, 2))
PE_KCS = {4, 8, 12, 20, 28}
CLS = {kc: ('act' if kc % 2 else ('pe' if kc in PE_KCS else 'saw'))
       for kc in range(KC)}
# (c0, n_chunks) with chunk stride 2; dtype = f16 for odd c0, fp8 for even
MASK_GROUPS = [(0, 4), (1, 4), (8, 4), (9, 4), (16, 4), (17, 4),
               (24, 4), (25, 4)]
VALP_RUNS = [(0, 1), (1, 1), (2, 2), (4, 2), (6, 2), (8, 4), (12, 4),
             (16, 4), (20, 4), (24, 4), (28, 4)]

# act-chunk mask multiplies alternate DVE / Pool (both SBUF-legal; Pool
# runs TT at 0.42x roofline so it only takes ~60%)
MSK_CYCLE = ('d', 'd', 'p', 'd', 'd', 'p', 'd', 'p')

_CACHE = {}


def _build_module():
    nc = bacc.Bacc("TRN2", target_bir_lowering=False, debug=False,
                   enable_asserts=True, num_devices=N_CORES)

    xkT = nc.dram_tensor("xkT", [E, S], F16, kind="ExternalInput").ap()
    xqT = nc.dram_tensor("xqT", [E, QLEN], F16, kind="ExternalInput").ap()
    valp = nc.dram_tensor("valp", [S, H * 65], BF16, kind="ExternalInput").ap()
    mbx = nc.dram_tensor("mbx", [S, QLEN], BF16, kind="ExternalInput").ap()
    mb8 = nc.dram_tensor("mb8", [S, QLEN], FP8, kind="ExternalInput").ap()
    g2 = nc.dram_tensor("g2", [128, 128], F16, kind="ExternalInput").ap()
    woe = nc.dram_tensor("woe", [E, E], F16, kind="ExternalInput").ap()
    bo_b = nc.dram_tensor("bo_b", [1, E], BF16, kind="ExternalInput").ap()
    eye_d = nc.dram_tensor("eye", [128, 128], F16, kind="ExternalInput").ap()
    i2_d = nc.dram_tensor("i2", [128, 256], FP8, kind="ExternalInput").ap()
    out = nc.dram_tensor("out", [QLEN, E], F32, kind="ExternalOutput").ap()

    with tile.TileContext(nc) as tc:
        _emit(tc, nc, xkT, xqT, valp, mbx, mb8, g2, woe, bo_b, eye_d, i2_d,
              out)

    nc.compile()
    return nc


def _emit(tc, nc, xkT, xqT, valp, mbx, mb8, g2, woe, bo_b, eye_d, i2_d, out):
    from contextlib import ExitStack
    ctx = ExitStack()
    const = ctx.enter_context(tc.tile_pool(name="const", bufs=1))
    kqp = ctx.enter_context(tc.tile_pool(name="kqp", bufs=1))
    xst = ctx.enter_context(tc.tile_pool(name="xst", bufs=2))
    ptp = ctx.enter_context(tc.tile_pool(name="pt", bufs=12))
    ctn_p = ctx.enter_context(tc.tile_pool(name="ctn", bufs=5))
    osb_p = ctx.enter_context(tc.tile_pool(name="osb", bufs=4))
    psb = ctx.enter_context(tc.tile_pool(name="psb", bufs=3, space="PSUM"))
    ctxp = ctx.enter_context(tc.tile_pool(name="ctxp", bufs=2, space="PSUM"))

    # ---------------- constants ----------------
    g2_sb = const.tile([128, 128], F16, tag="g2")
    nc.sync.dma_start(g2_sb, g2)
    i2 = const.tile([128, 256], FP8, tag="i2")
    nc.sync.dma_start(i2, i2_d)
    woe_sb = [const.tile([128, E], F16, tag=f"woe{pc}", name=f"woe{pc}")
              for pc in range(4)]
    bo_row = const.tile([1, E], BF16, tag="bo")
    ones1 = const.tile([1, 128], BF16, tag="ones1")
    nc.vector.memset(ones1, 1.0)
    biasB = const.tile([128, 1], F32, tag="biasB")
    nc.vector.memset(biasB, 0.0)
    biasC = const.tile([128, 1], F32, tag="biasC")
    nc.vector.memset(biasC, -240.0 * EXPSCALE)

    def load_late_consts():
        for pc in range(4):
            nc.sync.dma_start(woe_sb[pc], woe[pc * 128:(pc + 1) * 128, :])
        nc.sync.dma_start(bo_row, bo_b)

    # resident masks + V: one run-tile per contiguous same-class chunk run,
    # loaded by ONE grouped DMA each (HWDGE desc-gen would otherwise
    # serialize 64 transfers at ~600ns apiece). saw-chunk masks are fp8
    # (m*176); act-chunk masks f16 (keeps the DVE TT in 2x mode).
    mbx_res = {}
    mask_run_t = {}
    for (c0, k) in MASK_GROUPS:
        dt = FP8 if CLS[c0] != 'act' else BF16
        rt = const.tile([128, k * QLEN], dt, tag=f"mr{c0}", name=f"mr{c0}")
        mask_run_t[(c0, k)] = rt
        for j in range(k):
            mbx_res[c0 + 2 * j] = rt[:, j * QLEN:(j + 1) * QLEN]
    valp_run_t = {}
    valp_t = [None] * KC
    for (c0, k) in VALP_RUNS:
        rt = const.tile([128, k * H * 65], BF16, tag=f"vr{c0}", name=f"vr{c0}")
        valp_run_t[(c0, k)] = rt
        for j in range(k):
            valp_t[c0 + j] = rt[:, j * H * 65:(j + 1) * H * 65]

    def _grouped_dma(dst, src_ap, c0, k, row_elems):
        src = bass.AP(tensor=src_ap.tensor,
                      offset=src_ap.offset + c0 * 128 * row_elems,
                      ap=[[row_elems, 128], [128 * row_elems, k],
                          [1, row_elems]])
        nc.sync.dma_start(dst, src)

    def _mask_half_dma(c0, k, half):
        # load q-columns [half*512, half*512+512) of k stride-2 chunks in
        # one DMA: block 0 only touches the qw=0 halves, so splitting keeps
        # the critical first-block stream under its consumption rate.
        rt = mask_run_t[(c0, k)]
        src_ap = mb8 if CLS[c0] != 'act' else mbx
        dst = bass.AP(tensor=rt.tensor, offset=rt.offset + half * 512,
                      ap=[rt.ap[0], [QLEN, k], [1, 512]])
        src = bass.AP(tensor=src_ap.tensor,
                      offset=src_ap.offset + c0 * 128 * QLEN + half * 512,
                      ap=[[QLEN, 128], [2 * 128 * QLEN, k], [1, 512]])
        nc.sync.dma_start(dst, src)

    def _valp_pair_dma(c0, k, pair):
        # one pair's 130 V-columns for k chunks: the first attention blocks
        # only touch pair 0, so later pairs' V streams defer off the
        # critical startup window
        rt = valp_run_t[(c0, k)]
        dst = bass.AP(tensor=rt.tensor, offset=rt.offset + pair * 130,
                      ap=[rt.ap[0], [H * 65, k], [1, 130]])
        src = bass.AP(tensor=valp.tensor,
                      offset=valp.offset + c0 * 128 * H * 65 + pair * 130,
                      ap=[[H * 65, 128], [128 * H * 65, k], [1, 130]])
        nc.sync.dma_start(dst, src)

    def load_valp_pair(pair):
        for (c0, k) in VALP_RUNS:
            _valp_pair_dma(c0, k, pair)

    def load_kv_masks(lo, hi, half=0):
        # interleave mask runs and valp (pair 0) runs in first-need order
        evs = []
        for (c0, k) in MASK_GROUPS:
            if lo <= c0 < hi:
                evs.append((c0, 'm', (c0, k)))
        if half == 0:
            for (c0, k) in VALP_RUNS:
                if lo <= c0 < hi:
                    evs.append((c0 + 4, 'v', (c0, k)))  # needed ~LAG later
        evs.sort()
        for _, kind, (c0, k) in evs:
            if kind == 'm':
                _mask_half_dma(c0, k, half)
            else:
                _valp_pair_dma(c0, k, 0)

    # raw K tiles double as the score stationary operand; q projections
    qp2 = [kqp.tile([128, QLEN], F16, tag=f"qp2_{p}", name=f"qp2_{p}")
           for p in range(4)]
    concatT = [const.tile([128, QLEN], F16, tag=f"ct{p}", name=f"ct{p}")
               for p in range(4)]

    xs = {}

    def proj_load(pair):
        # HWDGE on the ACT queue: no Pool desc-gen cost, and deferred call
        # sites keep these transfers out of the resident-stream window.
        # xq first: the q-projection chain gates the first scores.
        xq = xst.tile([128, QLEN], F16, tag="xq", name=f"xq{pair}")
        nc.scalar.dma_start(xq, xqT[pair * 128:(pair + 1) * 128, :])
        xk0 = xst.tile([128, S // 2], F16, tag="xka", name=f"xka{pair}")
        nc.scalar.dma_start(xk0, xkT[pair * 128:(pair + 1) * 128, 0:S // 2])
        xk1 = xst.tile([128, S // 2], F16, tag="xkb", name=f"xkb{pair}")
        nc.scalar.dma_start(xk1, xkT[pair * 128:(pair + 1) * 128, S // 2:])
        xs[pair] = ((xk0, xk1), xq)

    def make_proj_chunk(pair, c):
        def run():
            _, xq = xs[pair]
            ps = psb.tile([128, 1024], F32, tag="ps", name=f"q{pair}_{c}")
            nc.tensor.matmul(ps[:, 0:512], lhsT=g2_sb,
                             rhs=xq[:, c * 512:(c + 1) * 512],
                             start=True, stop=True)
            nc.scalar.copy(qp2[pair][:, c * 512:(c + 1) * 512], ps[:, 0:512])
        return run

    # act-chunk mask engine alternation
    msk_i = [0]

    def msk_engine():
        e = MSK_CYCLE[msk_i[0] % len(MSK_CYCLE)]
        msk_i[0] += 1
        return nc.gpsimd if e == 'p' else nc.vector

    # ------------- flat software-pipelined attention stream -------------
    # 8 blocks x 64 half-units, one stream: scores(u) || exp(u) ||
    # AV(u-LAG) || deferred tail/transpose/proj/outproj works. AV matmuls of
    # block N drain while block N+1's scores stream, so PE never idles at
    # block boundaries; tail work is emitted a few units late so it never
    # parks at the head of an in-order engine queue.
    BLOCKS = [(p, qw) for p in range(4) for qw in range(2)]
    LAG = 18

    import heapq
    from collections import deque
    due = []         # heap of (due_u, seq, fn)
    seq_i = [0]
    u_now = [0]

    def sched(du, fn):
        heapq.heappush(due, (du, seq_i[0], fn))
        seq_i[0] += 1

    def pop_due(limit=2):
        n = 0
        while due and due[0][0] <= u_now[0] and n < limit:
            _, _, fn = heapq.heappop(due)
            fn()
            n += 1

    def scores_half(pair, qw, kc, h2, ps):
        """One [128,512] score matmul into bank h2 of the fused kc tile;
        'pe' chunks also fold the mask in via an fp8 DoubleRow identity."""
        (xk0, xk1), _ = xs[pair]
        xk = xk0 if kc < KC // 2 else xk1
        koff = 0 if kc < KC // 2 else S // 2
        dst = ps[:, h2 * 512:(h2 + 1) * 512]
        pe_cls = CLS[kc] == 'pe'
        nc.tensor.matmul(dst, lhsT=xk[h2 * 64:(h2 + 1) * 64,
                                      kc * 128 - koff:(kc + 1) * 128 - koff],
                         rhs=qp2[pair][h2 * 64:(h2 + 1) * 64,
                                       qw * 512:(qw + 1) * 512],
                         start=True, stop=not pe_cls)
        if pe_cls:
            # += 10 * (m*24) = 240*m into the bank (0.5 cycles/row)
            i2v = bass.AP(tensor=i2.tensor, offset=i2.offset,
                          ap=[i2.ap[0], [128, 2], [1, 128]])
            ms = mbx_res[kc][:, qw * 512:(qw + 1) * 512]
            mv = bass.AP(tensor=ms.tensor, offset=ms.offset,
                         ap=[ms.ap[0], [0, 2], [1, 512]])
            nc.tensor.matmul(dst, lhsT=i2v, rhs=mv, start=False, stop=True,
                             perf_mode=mybir.MatmulPerfMode.DoubleRow)

    def _h2view(t, half_elems=512):
        return bass.AP(tensor=t.tensor, offset=t.offset,
                       ap=[t.ap[0], [half_elems, 2], [1, half_elems]])

    def expmask_fused(pair, qw, kc, ps):
        """One elementwise op over both heads' banks [128,1024]; the mask
        slice broadcasts across h2 via a stride-0 middle dim."""
        ms = mbx_res[kc][:, qw * 512:(qw + 1) * 512]
        ms2 = bass.AP(tensor=ms.tensor, offset=ms.offset,
                      ap=[ms.ap[0], [0, 2], [1, 512]])
        if CLS[kc] == 'saw':
            pti = ptp.tile([128, 1024], I16, tag="pt",
                           name=f"pt{pair}_{qw}_{kc}")
            nc.vector.scalar_tensor_tensor(_h2view(pti), _h2view(ps), BB8,
                                           ms2, ALU.add, ALU.mult)
            return pti.bitcast(BF16)
        pt = ptp.tile([128, 1024], BF16, tag="pt", name=f"pt{pair}_{qw}_{kc}")
        bias = biasC if CLS[kc] == 'pe' else biasB
        nc.scalar.activation(pt, ps, AF.Exp, bias=bias, scale=EXPSCALE)
        if CLS[kc] == 'act':
            e = MSK_CYCLE[msk_i[0] % len(MSK_CYCLE)]
            msk_i[0] += 1
            if e == 'd':
                nc.vector.tensor_mul(_h2view(pt), _h2view(pt), ms2)
            else:
                # split: DVE takes bank A, Pool bank B (Pool TT is 0.42x
                # roofline; half-tiles keep it off the pt critical path)
                nc.vector.tensor_mul(pt[:, 0:512], pt[:, 0:512], ms)
                nc.gpsimd.tensor_mul(pt[:, 512:1024], pt[:, 512:1024], ms)
        return pt

    cx_t = {}        # (bi, h2) -> cx tile [128, 512] (one PSUM bank)
    ctn_t = {}       # (bi, h2, qt) -> normalized ctx tile

    def emit_av(bi, kc, h2, pt):
        pair, qw = BLOCKS[bi]
        cx = cx_t.get((bi, h2))
        if cx is None:
            cx = ctxp.tile([128, 512], F32, tag="cx", name=f"cx{bi}_{h2}")
            cx_t[(bi, h2)] = cx
        h = 2 * pair + h2
        # qt blocks at qt*65 share one 2KB psum bank; only the first matmul
        # sets start (its pending-zero covers the whole bank).
        for qt in range(4):
            nc.tensor.matmul(
                cx[:, qt * 65:qt * 65 + 65],
                lhsT=pt[:, qt * 128:qt * 128 + 128],
                rhs=valp_t[kc][:, h * 65:(h + 1) * 65],
                start=(kc == 0 and qt == 0), stop=(kc == KC - 1),
                skip_group_check=True)
        if kc == KC - 1:
            sched(u_now[0] + 2, make_tail(bi, h2))

    def make_tail(bi, h2):
        pair, qw = BLOCKS[bi]

        def tail():
            cx = cx_t[(bi, h2)]
            r = ctn_p.tile([128, 4], F32, tag="rec", name=f"rc{bi}_{h2}")
            dn = bass.AP(tensor=cx.tensor, offset=cx.offset + 64,
                         ap=[cx.ap[0], [65, 4]])
            with nc.allow_low_precision(reason="softmax denom reciprocal"):
                nc.vector.reciprocal(r, dn)
            # one fused normalize: (cx qt-blocks) * (r broadcast per qt).
            # Both h2 write one shared [128,512] tile with col layout
            # (qt*128 + h2*64 + d) so each qt slab is a PLAIN [128,128]
            # full-partition DMA transpose straight into concatT.
            if h2 == 0:
                t = ctn_p.tile([128, 512], F16, tag="ctn", name=f"cn{bi}")
                ctn_t[bi] = t
            t = ctn_t[bi]
            t3 = bass.AP(tensor=t.tensor, offset=t.offset + h2 * 64,
                         ap=[t.ap[0], [128, 4], [1, 64]])
            cx3 = bass.AP(tensor=cx.tensor, offset=cx.offset,
                          ap=[cx.ap[0], [65, 4], [1, 64]])
            r3 = bass.AP(tensor=r.tensor, offset=r.offset,
                         ap=[r.ap[0], [1, 4], [0, 64]])
            nc.vector.tensor_mul(t3, cx3, r3)
            if h2 == 1:
                late = bi == 7
                for qt in range(4):
                    sched(u_now[0] + (1 + qt if late else 3 + 2 * qt),
                          make_transp(bi, qt))
                    if late:
                        sched(u_now[0] + 2 + qt, make_outproj(4 + qt))
        return tail

    def make_transp(bi, qt):
        pair, qw = BLOCKS[bi]

        def go():
            # plain [128,128] xbar transpose: ctn qt-slab (cols h2*64+d)
            # -> concatT rows (h2*64+d), cols (qw*512 + qt*128 + q)
            src = ctn_t[bi][:, qt * 128:qt * 128 + 128]
            dst = concatT[pair][:, qw * 512 + qt * 128:qw * 512 + qt * 128 + 128]
            nc.sync.dma_start_transpose(dst, src)
        return go

    def make_outproj(qt):
        def go():
            opb = psb.tile([128, 1024], F32, tag="ps", name=f"op{qt}")
            op = opb[:, 0:512]
            for pc in range(4):
                nc.tensor.matmul(op,
                                 lhsT=concatT[pc][:, qt * 128:(qt + 1) * 128],
                                 rhs=woe_sb[pc],
                                 start=(pc == 0), stop=False)
            # bias via a K=1 ones-row matmul, then DMA straight from PSUM
            nc.tensor.matmul(op, lhsT=ones1, rhs=bo_row,
                             start=False, stop=True)
            osb = osb_p.tile([128, E], F32, tag="osb", name=f"osb{qt}")
            nc.scalar.copy(osb, op)
            nc.sync.dma_start(out[qt * 128:(qt + 1) * 128, :], osb)
        return go

    # ---------------- schedule ----------------
    proj_load(0)
    load_kv_masks(0, 8, half=0)
    make_proj_chunk(0, 0)()
    make_proj_chunk(0, 1)()
    load_kv_masks(8, KC, half=0)
    load_kv_masks(0, KC, half=1)
    load_late_consts()

    # deferred proj loads/chunks: pair p's x loads land mid-block (after the
    # resident-stream window); its q-proj chunks a block before first use.
    sched(40, lambda: proj_load(1))
    sched(46, lambda: load_valp_pair(1))
    sched(140, lambda: load_valp_pair(2))
    sched(270, lambda: load_valp_pair(3))
    sched(72, make_proj_chunk(1, 0))
    sched(76, make_proj_chunk(1, 1))
    sched(112, lambda: proj_load(2))
    sched(136, make_proj_chunk(2, 0))
    sched(140, make_proj_chunk(2, 1))
    sched(176, lambda: proj_load(3))
    sched(200, make_proj_chunk(3, 0))
    sched(204, make_proj_chunk(3, 1))
    # out-proj for q rows 0-511 after block 6's transposes; rest at drain.
    for i, qt in enumerate(range(4)):
        sched(482 + 4 * i, make_outproj(qt))

    pend = deque()
    for bi, (pair, qw) in enumerate(BLOCKS):
        for kc in range(KC):
            ps = psb.tile([128, 1024], F32, tag="ps",
                          name=f"ps{pair}_{qw}_{kc}")
            for h2 in range(2):
                scores_half(pair, qw, kc, h2, ps)
                if len(pend) >= LAG:
                    emit_av(*pend.popleft())
                    # fast-drain the previous block's trailing AVs so its
                    # tail (and the cx bank) frees before this block's
                    # first AV needs the ctxp slot
                    if pend and pend[0][0] != bi and len(pend) >= LAG - 4:
                        emit_av(*pend.popleft())
                if h2 == 1:
                    pt = expmask_fused(pair, qw, kc, ps)
                    pend.append((bi, kc, 0, pt[:, 0:512]))
                    pend.append((bi, kc, 1, pt[:, 512:1024]))
                pop_due()
                u_now[0] += 1
    while pend:
        emit_av(*pend.popleft())
        if pend:
            emit_av(*pend.popleft())
        pop_due()
        u_now[0] += 1
    # flush remaining deferred work (last tails, transposes, out-proj 4-7)
    while due:
        pop_due(limit=2)
        u_now[0] += 1

    ctx.close()


def _prep_inputs(key, query, value, mask, Wq, Wk, Wv, Wo, bo):
    f16 = np.float16
    bf16 = ml_dtypes.bfloat16
    key = np.asarray(key, np.float32)
    query = np.asarray(query, np.float32)
    value = np.asarray(value, np.float32)
    mask = np.asarray(mask)
    Wq = np.asarray(Wq, np.float32)
    Wk = np.asarray(Wk, np.float32)
    Wv = np.asarray(Wv, np.float32)
    Wo = np.asarray(Wo, np.float32)
    bo = np.asarray(bo, np.float32)

    # Wo_eff[e, h*64+u] = sum_d Wo[e, h*64+d] * Wv[d, u]
    wo_eff = np.empty((E, E), np.float32)
    for h in range(H):
        wo_eff[:, h * DH:(h + 1) * DH] = Wo[:, h * DH:(h + 1) * DH] @ Wv
    woe = np.ascontiguousarray(wo_eff.T).astype(f16)   # [(h,u), e]

    # G = Wq^T @ Wk folded q-side projection (scaled so saw-chunk masks are
    # fp8-exact m*176); blockdiag over the head pair
    G = (Wq.T @ Wk).astype(np.float32) * np.float32(QSCALE)
    g2 = np.zeros((128, 128), np.float32)
    g2[0:64, 0:64] = G
    g2[64:128, 64:128] = G

    m01 = (mask[0, 0] != 0).astype(np.float32).T  # [k, q] in {0,1}

    fp8 = ml_dtypes.float8_e4m3
    i2 = np.zeros((128, 256), np.float32)
    i2[:, 0:128] = np.eye(128) * 10.0
    common = {
        "g2": g2.astype(f16),
        "woe": woe,
        "bo_b": np.ascontiguousarray(bo[None, :]).astype(bf16),
        "eye": np.eye(128, dtype=np.float32).astype(f16),
        "i2": i2.astype(fp8),
    }
    per_b = {}
    for b in range(B):
        vp = np.ones((S, H, 65), np.float32)
        vp[:, :, :64] = value[b].reshape(S, H, DH)
        per_b[b] = {
            "xkT": np.ascontiguousarray(key[b].T).astype(f16),
            "valp": np.ascontiguousarray(vp.reshape(S, H * 65)).astype(bf16),
            "qT": query[b].T,
        }
    fp8 = ml_dtypes.float8_e4m3
    in_maps = []
    for c in range(N_CORES):
        b, qs = c // 4, (c % 4) * QLEN
        msl = np.ascontiguousarray(m01[:, qs:qs + QLEN])
        in_maps.append({
            "xkT": per_b[b]["xkT"],
            "xqT": np.ascontiguousarray(per_b[b]["qT"][:, qs:qs + QLEN]).astype(f16),
            "valp": per_b[b]["valp"],
            "mbx": msl.astype(bf16),
            "mb8": (msl * np.float32(V8)).astype(fp8),
            **common,
        })
    return in_maps


def get_module():
    if "nc" not in _CACHE:
        _CACHE["nc"] = _build_module()
    return _CACHE["nc"]


def kernel(key, query, value, mask, Wq, Wk, Wv, Wo, bo, **_):
    nc = get_module()
    in_maps = _prep_inputs(key, query, value, mask, Wq, Wk, Wv, Wo, bo)
    res = bass_utils.run_bass_kernel_spmd(
        nc, in_maps, core_ids=list(range(N_CORES)))
    full = np.empty((B, S, E), np.float32)
    for c in range(N_CORES):
        b, qs = c // 4, (c % 4) * QLEN
        full[b, qs:qs + QLEN, :] = res.results[c]["out"]
    return full


# revision 101
# speedup vs baseline: 1.0754x; 1.0029x over previous
"""Trainium2 Bass kernel for 8-head MultiHeadAttention (B=2, S=4096, E=512).

Sharding: 8 cores = 2 batches x 4 query-row chunks of 1024; each core runs
all 8 heads for its (batch, q-range) as 8 blocks (4 head-pairs x 2 query
windows) of 64 half-units, flattened into ONE software-pipelined stream.

Design:
- K-projection folded into Q on the host (G = Wq^T Wk, pre-scaled by
  QSCALE): scores = (x_q G) . x_k^T, so raw f16 K tiles are the score
  matmul's stationary operand and only a tiny Q-side projection remains.
  Wv is folded into Wo; the output bias rides as a K=1 ones-row matmul.
- Per kc, both heads' scores land in one [128,1024] two-bank PSUM tile;
  ONE fused elementwise op covers both: 'saw' chunks use a bf16-bits
  Schraudolph stt on DVE (i16 = round((s+BB8)*(24*m)) whose bits ARE the
  bf16 weights - mask folded free as the fp8 m*24 multiplicand); 'act'
  chunks use the ACT table exp with a post-exp bf16 mask multiply split
  DVE/Pool; 'pe' chunks use ACT exp with the mask pre-added into PSUM by
  an fp8 DoubleRow identity matmul (+240*m, exp bias -240*EXPSCALE).
  GPSIMD cannot touch PSUM, so Pool only gets SBUF-side mask work.
- Attention-value matmuls are flipped (pt stationary, V+ones moving, 65
  rows/matmul) and lag LAG half-units behind scores; AVs of block N drain
  while block N+1's scores stream, so PE never idles at block boundaries.
  The ones column lands the softmax denominator per partition.
- Tails (reciprocal + one broadcast-multiply normalize into a qt-major
  [128,512] tile) are deferred-scheduled a few units late so they never
  park at an in-order queue head; each qt slab then DMA-xbar-transposes
  straight into concatT (no PE transposes, no PSUM staging).
- Resident masks/V stream as grouped stride-2 multi-chunk DMAs (HWDGE
  desc-gen is ~600ns/DMA), split by q-half and V-columns by head-pair so
  block 0's critical window only loads what it reads.
"""
import sys
for _p in ('/root/.axon_site/_ro/trn_rl_repo', '/opt/trn_rl_repo'):
    if _p not in sys.path:
        sys.path.append(_p)

import numpy as np
import ml_dtypes

import concourse.bass as bass
import concourse.tile as tile
from concourse import bacc, mybir
from concourse import bass_utils

F32 = mybir.dt.float32
F16 = mybir.dt.float16
BF16 = mybir.dt.bfloat16
FP8 = mybir.dt.float8e4
I16 = mybir.dt.int16
AF = mybir.ActivationFunctionType
ALU = mybir.AluOpType

N_CORES = 8
B, S, E, H, DH = 2, 4096, 512, 8, 64
QLEN = S // 4           # 1024 q rows per core
KC = S // 128           # 32 k chunks

# bf16-bits Schraudolph: the dataset's extreme scores (|s| up to ~92, heavy
# product-normal tails) give weights up to e^11.4 = 9e4 > f16 max, so the
# weights (pt) live in bf16. bits = (QSCALE*s + BB8) * (24 * m): 24 is
# e4m3-exact, QSCALE = (128*log2(e)/8)/24 is folded into G on the host,
# BB8 calibrated for mean weight ratio 1 (rms sawtooth ~1.78%).
ABITS = 23.083120654223414      # 128*log2(e)/8
V8 = 24.0                       # fp8 mask multiplier
QSCALE = ABITS / V8             # host folds this into G
BB8 = 677.026428
EXPSCALE = 0.125 / QSCALE       # ACT exp scale on the rescaled scores

# per-kc classes (GPSIMD cannot touch PSUM, so exp lives on ACT+DVE only):
#  'saw': DVE Schraudolph stt, mask folded (fp8 m*176 operand)
#  'act': ACT table exp + post-exp f16 mask multiply (DVE or Pool, SBUF)
#  'pe':  ACT table exp, mask pre-added into the score PSUM by an fp8
#         DoubleRow identity matmul (i2=eye*1.5 against the m*176 fp8 mask
#         adds 264*m; exp bias -264*EXPSCALE turns masked scores into ~e-31)
# period-8 layout keeps runs short for engine smoothness and mask DMAs
# groupable by dtype run (mb8 for saw+pe, f16 for act).
# act = all odd kcs (ACT-exp runs stay <= 3 incl. the pe islands); pe on 4
# spread evens; saw on the other 12 evens. Odd/even interleave keeps every
# engine's per-unit load smooth AND leaves each dtype's chunks on an
# affine stride-2 grid so mask DMAs still group (4 chunks per transfer).
ACT_KCS = set(range(1, KC, 2))
PE_KCS = {4, 8, 12, 20, 28}
CLS = {kc: ('act' if kc % 2 else ('pe' if kc in PE_KCS else 'saw'))
       for kc in range(KC)}
# (c0, n_chunks) with chunk stride 2; dtype = f16 for odd c0, fp8 for even
MASK_GROUPS = [(0, 4), (1, 4), (8, 4), (9, 4), (16, 4), (17, 4),
               (24, 4), (25, 4)]
VALP_RUNS = [(0, 1), (1, 1), (2, 2), (4, 2), (6, 2), (8, 4), (12, 4),
             (16, 4), (20, 4), (24, 4), (28, 4)]

# act-chunk mask multiplies alternate DVE / Pool (both SBUF-legal; Pool
# runs TT at 0.42x roofline so it only takes ~60%)
MSK_CYCLE = ('d', 'd', 'p', 'd', 'd', 'p', 'd', 'p')

_CACHE = {}


def _build_module():
    nc = bacc.Bacc("TRN2", target_bir_lowering=False, debug=False,
                   enable_asserts=True, num_devices=N_CORES)

    xkT = nc.dram_tensor("xkT", [E, S], F16, kind="ExternalInput").ap()
    xqT = nc.dram_tensor("xqT", [E, QLEN], F16, kind="ExternalInput").ap()
    valp = nc.dram_tensor("valp", [S, H * 65], BF16, kind="ExternalInput").ap()
    mbx = nc.dram_tensor("mbx", [S, QLEN], BF16, kind="ExternalInput").ap()
    mb8 = nc.dram_tensor("mb8", [S, QLEN], FP8, kind="ExternalInput").ap()
    g2 = nc.dram_tensor("g2", [128, 128], F16, kind="ExternalInput").ap()
    woe = nc.dram_tensor("woe", [E, E], F16, kind="ExternalInput").ap()
    bo_b = nc.dram_tensor("bo_b", [1, E], BF16, kind="ExternalInput").ap()
    eye_d = nc.dram_tensor("eye", [128, 128], F16, kind="ExternalInput").ap()
    i2_d = nc.dram_tensor("i2", [128, 256], FP8, kind="ExternalInput").ap()
    out = nc.dram_tensor("out", [QLEN, E], F32, kind="ExternalOutput").ap()

    with tile.TileContext(nc) as tc:
        _emit(tc, nc, xkT, xqT, valp, mbx, mb8, g2, woe, bo_b, eye_d, i2_d,
              out)

    nc.compile()
    return nc


def _emit(tc, nc, xkT, xqT, valp, mbx, mb8, g2, woe, bo_b, eye_d, i2_d, out):
    from contextlib import ExitStack
    ctx = ExitStack()
    const = ctx.enter_context(tc.tile_pool(name="const", bufs=1))
    kqp = ctx.enter_context(tc.tile_pool(name="kqp", bufs=1))
    xst = ctx.enter_context(tc.tile_pool(name="xst", bufs=2))
    ptp = ctx.enter_context(tc.tile_pool(name="pt", bufs=12))
    ctn_p = ctx.enter_context(tc.tile_pool(name="ctn", bufs=5))
    osb_p = ctx.enter_context(tc.tile_pool(name="osb", bufs=4))
    psb = ctx.enter_context(tc.tile_pool(name="psb", bufs=3, space="PSUM"))
    ctxp = ctx.enter_context(tc.tile_pool(name="ctxp", bufs=2, space="PSUM"))

    # ---------------- constants ----------------
    g2_sb = const.tile([128, 128], F16, tag="g2")
    nc.sync.dma_start(g2_sb, g2)
    i2 = const.tile([128, 256], FP8, tag="i2")
    woe_sb = [const.tile([128, E], F16, tag=f"woe{pc}", name=f"woe{pc}")
              for pc in range(4)]
    bo_row = const.tile([1, E], BF16, tag="bo")
    ones1 = const.tile([1, 128], BF16, tag="ones1")
    nc.vector.memset(ones1, 1.0)
    biasB = const.tile([128, 1], F32, tag="biasB")
    nc.vector.memset(biasB, 0.0)
    biasC = const.tile([128, 1], F32, tag="biasC")
    nc.vector.memset(biasC, -240.0 * EXPSCALE)
    # warm the ACT exp table during the startup DMA window (the 1283ns
    # table load would otherwise delay the first real exp)
    warm = const.tile([128, 1], F32, tag="warm")
    nc.scalar.activation(warm, biasB, AF.Exp, bias=biasB, scale=1.0)

    def load_late_consts():
        for pc in range(4):
            nc.sync.dma_start(woe_sb[pc], woe[pc * 128:(pc + 1) * 128, :])
        nc.sync.dma_start(bo_row, bo_b)

    # resident masks + V: one run-tile per contiguous same-class chunk run,
    # loaded by ONE grouped DMA each (HWDGE desc-gen would otherwise
    # serialize 64 transfers at ~600ns apiece). saw-chunk masks are fp8
    # (m*176); act-chunk masks f16 (keeps the DVE TT in 2x mode).
    mbx_res = {}
    mask_run_t = {}
    for (c0, k) in MASK_GROUPS:
        dt = FP8 if CLS[c0] != 'act' else BF16
        rt = const.tile([128, k * QLEN], dt, tag=f"mr{c0}", name=f"mr{c0}")
        mask_run_t[(c0, k)] = rt
        for j in range(k):
            mbx_res[c0 + 2 * j] = rt[:, j * QLEN:(j + 1) * QLEN]
    valp_run_t = {}
    valp_t = [None] * KC
    for (c0, k) in VALP_RUNS:
        rt = const.tile([128, k * H * 65], BF16, tag=f"vr{c0}", name=f"vr{c0}")
        valp_run_t[(c0, k)] = rt
        for j in range(k):
            valp_t[c0 + j] = rt[:, j * H * 65:(j + 1) * H * 65]

    def _grouped_dma(dst, src_ap, c0, k, row_elems):
        src = bass.AP(tensor=src_ap.tensor,
                      offset=src_ap.offset + c0 * 128 * row_elems,
                      ap=[[row_elems, 128], [128 * row_elems, k],
                          [1, row_elems]])
        nc.sync.dma_start(dst, src)

    def _mask_half_dma(c0, k, half):
        # load q-columns [half*512, half*512+512) of k stride-2 chunks in
        # one DMA: block 0 only touches the qw=0 halves, so splitting keeps
        # the critical first-block stream under its consumption rate.
        rt = mask_run_t[(c0, k)]
        src_ap = mb8 if CLS[c0] != 'act' else mbx
        dst = bass.AP(tensor=rt.tensor, offset=rt.offset + half * 512,
                      ap=[rt.ap[0], [QLEN, k], [1, 512]])
        src = bass.AP(tensor=src_ap.tensor,
                      offset=src_ap.offset + c0 * 128 * QLEN + half * 512,
                      ap=[[QLEN, 128], [2 * 128 * QLEN, k], [1, 512]])
        nc.sync.dma_start(dst, src)

    def _valp_pair_dma(c0, k, pair):
        # one pair's 130 V-columns for k chunks: the first attention blocks
        # only touch pair 0, so later pairs' V streams defer off the
        # critical startup window
        rt = valp_run_t[(c0, k)]
        dst = bass.AP(tensor=rt.tensor, offset=rt.offset + pair * 130,
                      ap=[rt.ap[0], [H * 65, k], [1, 130]])
        src = bass.AP(tensor=valp.tensor,
                      offset=valp.offset + c0 * 128 * H * 65 + pair * 130,
                      ap=[[H * 65, 128], [128 * H * 65, k], [1, 130]])
        nc.sync.dma_start(dst, src)

    def load_valp_pair(pair):
        for (c0, k) in VALP_RUNS:
            _valp_pair_dma(c0, k, pair)

    def load_kv_masks(lo, hi, half=0):
        # interleave mask runs and valp (pair 0) runs in first-need order
        evs = []
        for (c0, k) in MASK_GROUPS:
            if lo <= c0 < hi:
                evs.append((c0, 'm', (c0, k)))
        if half == 0:
            for (c0, k) in VALP_RUNS:
                if lo <= c0 < hi:
                    evs.append((c0 + 4, 'v', (c0, k)))  # needed ~LAG later
        evs.sort()
        for _, kind, (c0, k) in evs:
            if kind == 'm':
                _mask_half_dma(c0, k, half)
            else:
                _valp_pair_dma(c0, k, 0)

    # raw K tiles double as the score stationary operand; q projections
    qp2 = [kqp.tile([128, QLEN], F16, tag=f"qp2_{p}", name=f"qp2_{p}")
           for p in range(4)]
    concatT = [const.tile([128, QLEN], F16, tag=f"ct{p}", name=f"ct{p}")
               for p in range(4)]

    xs = {}

    def proj_load(pair):
        # HWDGE on the ACT queue: no Pool desc-gen cost, and deferred call
        # sites keep these transfers out of the resident-stream window.
        # xq first: the q-projection chain gates the first scores.
        xq = xst.tile([128, QLEN], F16, tag="xq", name=f"xq{pair}")
        nc.scalar.dma_start(xq, xqT[pair * 128:(pair + 1) * 128, :])
        xk0 = xst.tile([128, S // 2], F16, tag="xka", name=f"xka{pair}")
        nc.scalar.dma_start(xk0, xkT[pair * 128:(pair + 1) * 128, 0:S // 2])
        xk1 = xst.tile([128, S // 2], F16, tag="xkb", name=f"xkb{pair}")
        nc.scalar.dma_start(xk1, xkT[pair * 128:(pair + 1) * 128, S // 2:])
        xs[pair] = ((xk0, xk1), xq)

    def make_proj_chunk(pair, c):
        def run():
            _, xq = xs[pair]
            ps = psb.tile([128, 1024], F32, tag="ps", name=f"q{pair}_{c}")
            nc.tensor.matmul(ps[:, 0:512], lhsT=g2_sb,
                             rhs=xq[:, c * 512:(c + 1) * 512],
                             start=True, stop=True)
            nc.scalar.copy(qp2[pair][:, c * 512:(c + 1) * 512], ps[:, 0:512])
        return run

    # act-chunk mask engine alternation
    msk_i = [0]

    def msk_engine():
        e = MSK_CYCLE[msk_i[0] % len(MSK_CYCLE)]
        msk_i[0] += 1
        return nc.gpsimd if e == 'p' else nc.vector

    # ------------- flat software-pipelined attention stream -------------
    # 8 blocks x 64 half-units, one stream: scores(u) || exp(u) ||
    # AV(u-LAG) || deferred tail/transpose/proj/outproj works. AV matmuls of
    # block N drain while block N+1's scores stream, so PE never idles at
    # block boundaries; tail work is emitted a few units late so it never
    # parks at the head of an in-order engine queue.
    BLOCKS = [(p, qw) for p in range(4) for qw in range(2)]
    LAG = 18

    import heapq
    from collections import deque
    due = []         # heap of (due_u, seq, fn)
    seq_i = [0]
    u_now = [0]

    def sched(du, fn):
        heapq.heappush(due, (du, seq_i[0], fn))
        seq_i[0] += 1

    def pop_due(limit=2):
        n = 0
        while due and due[0][0] <= u_now[0] and n < limit:
            _, _, fn = heapq.heappop(due)
            fn()
            n += 1

    def scores_half(pair, qw, kc, h2, ps):
        """One [128,512] score matmul into bank h2 of the fused kc tile;
        'pe' chunks also fold the mask in via an fp8 DoubleRow identity."""
        (xk0, xk1), _ = xs[pair]
        xk = xk0 if kc < KC // 2 else xk1
        koff = 0 if kc < KC // 2 else S // 2
        dst = ps[:, h2 * 512:(h2 + 1) * 512]
        pe_cls = CLS[kc] == 'pe'
        nc.tensor.matmul(dst, lhsT=xk[h2 * 64:(h2 + 1) * 64,
                                      kc * 128 - koff:(kc + 1) * 128 - koff],
                         rhs=qp2[pair][h2 * 64:(h2 + 1) * 64,
                                       qw * 512:(qw + 1) * 512],
                         start=True, stop=not pe_cls)
        if pe_cls:
            # += 10 * (m*24) = 240*m into the bank (0.5 cycles/row)
            i2v = bass.AP(tensor=i2.tensor, offset=i2.offset,
                          ap=[i2.ap[0], [128, 2], [1, 128]])
            ms = mbx_res[kc][:, qw * 512:(qw + 1) * 512]
            mv = bass.AP(tensor=ms.tensor, offset=ms.offset,
                         ap=[ms.ap[0], [0, 2], [1, 512]])
            nc.tensor.matmul(dst, lhsT=i2v, rhs=mv, start=False, stop=True,
                             perf_mode=mybir.MatmulPerfMode.DoubleRow)

    def _h2view(t, half_elems=512):
        return bass.AP(tensor=t.tensor, offset=t.offset,
                       ap=[t.ap[0], [half_elems, 2], [1, half_elems]])

    def expmask_fused(pair, qw, kc, ps):
        """One elementwise op over both heads' banks [128,1024]; the mask
        slice broadcasts across h2 via a stride-0 middle dim."""
        ms = mbx_res[kc][:, qw * 512:(qw + 1) * 512]
        ms2 = bass.AP(tensor=ms.tensor, offset=ms.offset,
                      ap=[ms.ap[0], [0, 2], [1, 512]])
        if CLS[kc] == 'saw':
            pti = ptp.tile([128, 1024], I16, tag="pt",
                           name=f"pt{pair}_{qw}_{kc}")
            nc.vector.scalar_tensor_tensor(_h2view(pti), _h2view(ps), BB8,
                                           ms2, ALU.add, ALU.mult)
            return pti.bitcast(BF16)
        pt = ptp.tile([128, 1024], BF16, tag="pt", name=f"pt{pair}_{qw}_{kc}")
        bias = biasC if CLS[kc] == 'pe' else biasB
        nc.scalar.activation(pt, ps, AF.Exp, bias=bias, scale=EXPSCALE)
        if CLS[kc] == 'act':
            e = MSK_CYCLE[msk_i[0] % len(MSK_CYCLE)]
            msk_i[0] += 1
            if e == 'd':
                nc.vector.tensor_mul(_h2view(pt), _h2view(pt), ms2)
            else:
                # split: DVE takes bank A, Pool bank B (Pool TT is 0.42x
                # roofline; half-tiles keep it off the pt critical path)
                nc.vector.tensor_mul(pt[:, 0:512], pt[:, 0:512], ms)
                nc.gpsimd.tensor_mul(pt[:, 512:1024], pt[:, 512:1024], ms)
        return pt

    cx_t = {}        # (bi, h2) -> cx tile [128, 512] (one PSUM bank)
    ctn_t = {}       # (bi, h2, qt) -> normalized ctx tile

    def emit_av(bi, kc, h2, pt):
        pair, qw = BLOCKS[bi]
        cx = cx_t.get((bi, h2))
        if cx is None:
            cx = ctxp.tile([128, 512], F32, tag="cx", name=f"cx{bi}_{h2}")
            cx_t[(bi, h2)] = cx
        h = 2 * pair + h2
        # qt blocks at qt*65 share one 2KB psum bank; only the first matmul
        # sets start (its pending-zero covers the whole bank).
        for qt in range(4):
            nc.tensor.matmul(
                cx[:, qt * 65:qt * 65 + 65],
                lhsT=pt[:, qt * 128:qt * 128 + 128],
                rhs=valp_t[kc][:, h * 65:(h + 1) * 65],
                start=(kc == 0 and qt == 0), stop=(kc == KC - 1),
                skip_group_check=True)
        if kc == KC - 1:
            sched(u_now[0] + 2, make_tail(bi, h2))

    def make_tail(bi, h2):
        pair, qw = BLOCKS[bi]

        def tail():
            cx = cx_t[(bi, h2)]
            r = ctn_p.tile([128, 4], F32, tag="rec", name=f"rc{bi}_{h2}")
            dn = bass.AP(tensor=cx.tensor, offset=cx.offset + 64,
                         ap=[cx.ap[0], [65, 4]])
            with nc.allow_low_precision(reason="softmax denom reciprocal"):
                nc.vector.reciprocal(r, dn)
            # one fused normalize: (cx qt-blocks) * (r broadcast per qt).
            # Both h2 write one shared [128,512] tile with col layout
            # (qt*128 + h2*64 + d) so each qt slab is a PLAIN [128,128]
            # full-partition DMA transpose straight into concatT.
            if h2 == 0:
                t = ctn_p.tile([128, 512], F16, tag="ctn", name=f"cn{bi}")
                ctn_t[bi] = t
            t = ctn_t[bi]
            t3 = bass.AP(tensor=t.tensor, offset=t.offset + h2 * 64,
                         ap=[t.ap[0], [128, 4], [1, 64]])
            cx3 = bass.AP(tensor=cx.tensor, offset=cx.offset,
                          ap=[cx.ap[0], [65, 4], [1, 64]])
            r3 = bass.AP(tensor=r.tensor, offset=r.offset,
                         ap=[r.ap[0], [1, 4], [0, 64]])
            nc.vector.tensor_mul(t3, cx3, r3)
            if h2 == 1:
                late = bi == 7
                for qt in range(4):
                    sched(u_now[0] + (1 + qt if late else 3 + 2 * qt),
                          make_transp(bi, qt))
                    if late:
                        sched(u_now[0] + 2 + qt, make_outproj(4 + qt))
        return tail

    def make_transp(bi, qt):
        pair, qw = BLOCKS[bi]

        def go():
            # plain [128,128] xbar transpose: ctn qt-slab (cols h2*64+d)
            # -> concatT rows (h2*64+d), cols (qw*512 + qt*128 + q)
            src = ctn_t[bi][:, qt * 128:qt * 128 + 128]
            dst = concatT[pair][:, qw * 512 + qt * 128:qw * 512 + qt * 128 + 128]
            nc.sync.dma_start_transpose(dst, src)
        return go

    def make_outproj(qt):
        def go():
            opb = psb.tile([128, 1024], F32, tag="ps", name=f"op{qt}")
            op = opb[:, 0:512]
            for pc in range(4):
                nc.tensor.matmul(op,
                                 lhsT=concatT[pc][:, qt * 128:(qt + 1) * 128],
                                 rhs=woe_sb[pc],
                                 start=(pc == 0), stop=False)
            # bias via a K=1 ones-row matmul, then DMA straight from PSUM
            nc.tensor.matmul(op, lhsT=ones1, rhs=bo_row,
                             start=False, stop=True)
            osb = osb_p.tile([128, E], F32, tag="osb", name=f"osb{qt}")
            nc.scalar.copy(osb, op)
            nc.sync.dma_start(out[qt * 128:(qt + 1) * 128, :], osb)
        return go

    # ---------------- schedule ----------------
    proj_load(0)
    load_kv_masks(0, 2, half=0)
    nc.sync.dma_start(i2, i2_d)
    load_kv_masks(2, 8, half=0)
    make_proj_chunk(0, 0)()
    make_proj_chunk(0, 1)()
    load_kv_masks(8, KC, half=0)
    load_kv_masks(0, KC, half=1)
    load_late_consts()

    # deferred proj loads/chunks: pair p's x loads land mid-block (after the
    # resident-stream window); its q-proj chunks a block before first use.
    # pair-1 loads are needed by block 2 (~u128) but must stay clear of
    # the qw=1 mask stream that block 1 consumes through ~u110
    sched(100, lambda: proj_load(1))
    sched(112, lambda: load_valp_pair(1))
    sched(140, lambda: load_valp_pair(2))
    sched(270, lambda: load_valp_pair(3))
    sched(104, make_proj_chunk(1, 0))
    sched(108, make_proj_chunk(1, 1))
    sched(112, lambda: proj_load(2))
    sched(136, make_proj_chunk(2, 0))
    sched(140, make_proj_chunk(2, 1))
    sched(176, lambda: proj_load(3))
    sched(200, make_proj_chunk(3, 0))
    sched(204, make_proj_chunk(3, 1))
    # out-proj for q rows 0-511 after block 6's transposes; rest at drain.
    for i, qt in enumerate(range(4)):
        sched(482 + 4 * i, make_outproj(qt))

    pend = deque()
    for bi, (pair, qw) in enumerate(BLOCKS):
        for kc in range(KC):
            ps = psb.tile([128, 1024], F32, tag="ps",
                          name=f"ps{pair}_{qw}_{kc}")
            for h2 in range(2):
                scores_half(pair, qw, kc, h2, ps)
                if len(pend) >= LAG:
                    emit_av(*pend.popleft())
                    # fast-drain the previous block's trailing AVs so its
                    # tail (and the cx bank) frees before this block's
                    # first AV needs the ctxp slot
                    if pend and pend[0][0] != bi and len(pend) >= LAG - 4:
                        emit_av(*pend.popleft())
                if h2 == 1:
                    pt = expmask_fused(pair, qw, kc, ps)
                    pend.append((bi, kc, 0, pt[:, 0:512]))
                    pend.append((bi, kc, 1, pt[:, 512:1024]))
                pop_due()
                u_now[0] += 1
    while pend:
        emit_av(*pend.popleft())
        if pend:
            emit_av(*pend.popleft())
        pop_due()
        u_now[0] += 1
    # flush remaining deferred work (last tails, transposes, out-proj 4-7)
    while due:
        pop_due(limit=2)
        u_now[0] += 1

    ctx.close()


def _prep_inputs(key, query, value, mask, Wq, Wk, Wv, Wo, bo):
    f16 = np.float16
    bf16 = ml_dtypes.bfloat16
    key = np.asarray(key, np.float32)
    query = np.asarray(query, np.float32)
    value = np.asarray(value, np.float32)
    mask = np.asarray(mask)
    Wq = np.asarray(Wq, np.float32)
    Wk = np.asarray(Wk, np.float32)
    Wv = np.asarray(Wv, np.float32)
    Wo = np.asarray(Wo, np.float32)
    bo = np.asarray(bo, np.float32)

    # Wo_eff[e, h*64+u] = sum_d Wo[e, h*64+d] * Wv[d, u]
    wo_eff = np.empty((E, E), np.float32)
    for h in range(H):
        wo_eff[:, h * DH:(h + 1) * DH] = Wo[:, h * DH:(h + 1) * DH] @ Wv
    woe = np.ascontiguousarray(wo_eff.T).astype(f16)   # [(h,u), e]

    # G = Wq^T @ Wk folded q-side projection (scaled so saw-chunk masks are
    # fp8-exact m*176); blockdiag over the head pair
    G = (Wq.T @ Wk).astype(np.float32) * np.float32(QSCALE)
    g2 = np.zeros((128, 128), np.float32)
    g2[0:64, 0:64] = G
    g2[64:128, 64:128] = G

    m01 = (mask[0, 0] != 0).astype(np.float32).T  # [k, q] in {0,1}

    fp8 = ml_dtypes.float8_e4m3
    i2 = np.zeros((128, 256), np.float32)
    i2[:, 0:128] = np.eye(128) * 10.0
    common = {
        "g2": g2.astype(f16),
        "woe": woe,
        "bo_b": np.ascontiguousarray(bo[None, :]).astype(bf16),
        "eye": np.eye(128, dtype=np.float32).astype(f16),
        "i2": i2.astype(fp8),
    }
    per_b = {}
    for b in range(B):
        vp = np.ones((S, H, 65), np.float32)
        vp[:, :, :64] = value[b].reshape(S, H, DH)
        per_b[b] = {
            "xkT": np.ascontiguousarray(key[b].T).astype(f16),
            "valp": np.ascontiguousarray(vp.reshape(S, H * 65)).astype(bf16),
            "qT": query[b].T,
        }
    fp8 = ml_dtypes.float8_e4m3
    in_maps = []
    for c in range(N_CORES):
        b, qs = c // 4, (c % 4) * QLEN
        msl = np.ascontiguousarray(m01[:, qs:qs + QLEN])
        in_maps.append({
            "xkT": per_b[b]["xkT"],
            "xqT": np.ascontiguousarray(per_b[b]["qT"][:, qs:qs + QLEN]).astype(f16),
            "valp": per_b[b]["valp"],
            "mbx": msl.astype(bf16),
            "mb8": (msl * np.float32(V8)).astype(fp8),
            **common,
        })
    return in_maps


def get_module():
    if "nc" not in _CACHE:
        _CACHE["nc"] = _build_module()
    return _CACHE["nc"]


def kernel(key, query, value, mask, Wq, Wk, Wv, Wo, bo, **_):
    nc = get_module()
    in_maps = _prep_inputs(key, query, value, mask, Wq, Wk, Wv, Wo, bo)
    res = bass_utils.run_bass_kernel_spmd(
        nc, in_maps, core_ids=list(range(N_CORES)))
    full = np.empty((B, S, E), np.float32)
    for c in range(N_CORES):
        b, qs = c // 4, (c % 4) * QLEN
        full[b, qs:qs + QLEN, :] = res.results[c]["out"]
    return full
